# revision 1
# baseline (speedup 1.0000x reference)
"""EquivSetGNN forward on 8 Trainium2 NeuronCores (Bass/Tile).

Sharding: nodes (and their incident nnz entries, src-partitioned) are split
evenly across 8 cores. Per layer:
  V->E: each core gathers h[src] rows for its (dst-sorted) entries via
        dma_gather, segment-sums them per 128-wide dst window using PE
        matmuls with DVE-built one-hot selection matrices (PSUM
        accumulation), scales by 1/deg(dst) during flush, producing a
        partial Xe table; AllReduce across cores yields the full Xe.
  E->V: gathers Xe[dst] rows for its (src-window-ordered) entries, same
        one-hot matmul segment-sum into node windows scaled by 1/deg(src).
Dense MLP phases run in feature-transposed layout [64, n] per core.
Readout: per-graph sums via one-hot matmuls + small AllReduce + 1/count.
"""
import sys

sys.path.insert(0, "/opt/trn_rl_repo")

import ml_dtypes
import numpy as np

import concourse.bass as bass
import concourse.bacc as bacc
import concourse.mybir as mybir
import concourse.tile as tile
from concourse.bass_utils import run_bass_kernel_spmd
from concourse.library_config import mlp as mlp_lib
from concourse.masks import make_identity
from contextlib import ExitStack

F32 = mybir.dt.float32
BF16 = mybir.dt.bfloat16
I16 = mybir.dt.int16
AF = mybir.ActivationFunctionType
ALU = mybir.AluOpType


class Cfg:
    def __init__(self, N=100000, E=50000, FT=128, HID=64, CLS_H=64, NCLS=32,
                 NGRAPH=256, NLAYER=2, NCORES=8, EHALF=32768):
        self.N, self.E, self.FT, self.HID = N, E, FT, HID
        self.CLS_H, self.NCLS, self.NGRAPH, self.NLAYER = CLS_H, NCLS, NGRAPH, NLAYER
        self.NCORES = NCORES
        self.EHALF = EHALF  # int16 split point for gathers from Xe
        assert N % NCORES == 0
        self.NLOC = N // NCORES
        self.NW = -(-self.NLOC // 128)       # node windows per core
        self.EW = -(-E // 128)               # edge windows
        self.GW = -(-NGRAPH // 128)          # graph windows
        self.EPAD = self.EW * 128


def _ceil(a, b):
    return -(-a // b)


def _wrap16(idx):
    """flat idx array -> [128, L/16] int16 wrapped layout (replicated x8)."""
    a = np.asarray(idx, np.int16).reshape(-1, 16).T
    return np.ascontiguousarray(np.tile(a, (8, 1)))


def _gath_layout(vals, fill, dtype):
    """flat [L] -> [128, L/128] gathered layout (entry k at [k%128, k//128])."""
    L = len(vals)
    assert L % 128 == 0
    return np.ascontiguousarray(np.asarray(vals, dtype).reshape(L // 128, 128).T)


def prep(cfg, X, v2e_src, v2e_dst, all_batch):
    """Host preprocessing -> (shared_structure, per_core_input_maps)."""
    c = cfg
    src = np.asarray(v2e_src, np.int64)
    dst = np.asarray(v2e_dst, np.int64)
    batch = np.asarray(all_batch, np.int64)

    d_deg = np.bincount(dst, minlength=c.E).astype(np.float32)
    c_deg = np.bincount(src, minlength=c.N).astype(np.float32)
    recip_d = (1.0 / np.maximum(d_deg, 1.0)).astype(np.float32)
    recip_c = (1.0 / np.maximum(c_deg, 1.0)).astype(np.float32)

    cores = []
    for ci in range(c.NCORES):
        lo, hi = np.searchsorted(src, [c.NLOC * ci, c.NLOC * (ci + 1)])
        s = (src[lo:hi] - c.NLOC * ci).astype(np.int64)
        e = dst[lo:hi]
        cores.append((s, e))

    # ---- V->E stream (dst-sorted), per edge-window block counts ----
    cntA = np.zeros((c.NCORES, c.EW), np.int64)
    coreA = []
    for ci, (s, e) in enumerate(cores):
        order = np.argsort(e, kind="stable")
        sA, eA = s[order], e[order]
        win = eA >> 7
        cntA[ci] = np.bincount(win, minlength=c.EW)
        coreA.append((sA, eA, win))
    BA = _ceil(cntA.max(axis=0), 1)
    BA = -(-BA // 128)  # blocks per window (shared)
    BA = np.maximum(BA, 0)
    capA = BA * 128
    offA = np.concatenate([[0], np.cumsum(capA)])
    LA = int(offA[-1])
    nblkA = LA // 128

    # ---- E->V stream (node-window ordered, L then H per window) ----
    cntL = np.zeros((c.NCORES, c.NW), np.int64)
    cntH = np.zeros((c.NCORES, c.NW), np.int64)
    coreB = []
    for ci, (s, e) in enumerate(cores):
        m = s >> 7
        isH = (e >= c.EHALF).astype(np.int64)
        order = np.argsort(m * 2 + isH, kind="stable")
        sB, eB, mB, hB = s[order], e[order], m[order], isH[order]
        cntL[ci] = np.bincount(mB[hB == 0], minlength=c.NW)
        cntH[ci] = np.bincount(mB[hB == 1], minlength=c.NW)
        coreB.append((sB, eB, mB, hB))
    BL = -(-cntL.max(axis=0) // 128)
    BH = -(-cntH.max(axis=0) // 128)
    capL, capH = BL * 128, BH * 128
    # L parts of all windows first, then all H parts
    offL = np.concatenate([[0], np.cumsum(capL)])[:-1]
    LBL = int(capL.sum())
    offH = LBL + np.concatenate([[0], np.cumsum(capH)])[:-1]
    LB = LBL + int(capH.sum())
    nblkB = LB // 128
    nblkBL = LBL // 128

    shared = dict(BA=BA, BL=BL, BH=BH, LA=LA, LB=LB, nblkA=nblkA, nblkB=nblkB,
                  nblkBL=nblkBL, offA=offA, offL=offL, offH=offH)

    # graph counts
    gcnt = np.bincount(batch, minlength=c.NGRAPH).astype(np.float32)
    recip_g = (1.0 / np.maximum(gcnt, 1.0)).astype(np.float32)
    recip_g_win = np.zeros((128, c.GW), np.float32)
    for g in range(c.NGRAPH):
        recip_g_win[g % 128, g // 128] = recip_g[g]

    in_maps = []
    for ci in range(c.NCORES):
        # V->E placement
        sA, eA, winA = coreA[ci]
        startsA = np.searchsorted(winA, np.arange(c.EW))
        place = offA[winA] + (np.arange(len(winA)) - startsA[winA])
        gidxA = np.zeros(LA, np.int64)
        idsA = np.full(LA, -1.0, np.float32)
        gidxA[place] = sA
        idsA[place] = (eA - (winA << 7)).astype(np.float32)

        # E->V placement
        sB, eB, mB, hB = coreB[ci]
        keyB = mB * 2 + hB
        startsB = np.searchsorted(keyB, np.arange(2 * c.NW))
        base = np.where(hB == 0, offL[mB], offH[mB])
        place = base + (np.arange(len(keyB)) - startsB[keyB])
        gidxB = np.zeros(LB, np.int64)
        idsB = np.full(LB, -1.0, np.float32)
        gidxB[place] = np.where(hB == 0, eB, eB - c.EHALF)
        idsB[place] = (sB - (mB << 7)).astype(np.float32)

        # per-node arrays in window layout [128, NW]
        npad = c.NW * 128
        cw = np.zeros(npad, np.float32)
        cw[:c.NLOC] = recip_c[c.NLOC * ci: c.NLOC * (ci + 1)]
        recip_c_win = np.ascontiguousarray(cw.reshape(c.NW, 128).T)
        mw = np.zeros(npad, np.float32)
        mw[:c.NLOC] = (c_deg[c.NLOC * ci: c.NLOC * (ci + 1)] > 0).astype(np.float32)
        mask_win = np.ascontiguousarray(mw.reshape(c.NW, 128).T)
        bw = np.full(npad, -1.0, np.float32)
        bw[:c.NLOC] = batch[c.NLOC * ci: c.NLOC * (ci + 1)].astype(np.float32)
        ids_g = np.ascontiguousarray(bw.reshape(c.NW, 128).T)
        bw1 = np.where(bw < 0, -1.0, bw - 128.0).astype(np.float32)
        ids_g1 = np.ascontiguousarray(bw1.reshape(c.NW, 128).T)

        dw = np.zeros(c.EW * 128, np.float32)
        dw[:c.E] = recip_d
        recip_d_win = np.ascontiguousarray(dw.reshape(c.EW, 128).T)

        iota2d = np.broadcast_to(np.arange(128).astype(ml_dtypes.bfloat16), (128, 128))
        iota2d = np.ascontiguousarray(iota2d.reshape(128, 1, 128))

        m = {
            "Xs": np.ascontiguousarray(X[c.NLOC * ci: c.NLOC * (ci + 1)]).astype(np.float32),
            "gidxA": _wrap16(gidxA), "idsA": _gath_layout(idsA, -1.0, ml_dtypes.bfloat16),
            "gidxB": _wrap16(gidxB), "idsB": _gath_layout(idsB, -1.0, ml_dtypes.bfloat16),
            "recip_c_win": recip_c_win, "mask_win": mask_win,
            "recip_d_win": recip_d_win, "ids_g": ids_g, "ids_g1": ids_g1,
            "recip_g_win": recip_g_win, "iota2d": iota2d,
        }
        in_maps.append(m)
    return shared, in_maps


def build(cfg, sh, weights_shapes, debug_taps=False):
    """Build the SPMD Bass program. weights_shapes: dict name->shape."""
    c = cfg
    nc = bacc.Bacc("TRN2", debug=False, num_swdge_queues=1)
    HID = c.HID

    # ---------- I/O ----------
    Xs = nc.declare_dram_parameter("Xs", [c.NLOC, c.FT], F32, isOutput=False)
    gidxA_d = nc.declare_dram_parameter("gidxA", [128, sh["LA"] // 16], I16, isOutput=False)
    idsA_d = nc.declare_dram_parameter("idsA", [128, sh["nblkA"]], BF16, isOutput=False)
    gidxB_d = nc.declare_dram_parameter("gidxB", [128, sh["LB"] // 16], I16, isOutput=False)
    idsB_d = nc.declare_dram_parameter("idsB", [128, sh["nblkB"]], BF16, isOutput=False)
    recip_c_d = nc.declare_dram_parameter("recip_c_win", [128, c.NW], F32, isOutput=False)
    mask_d = nc.declare_dram_parameter("mask_win", [128, c.NW], F32, isOutput=False)
    recip_d_d = nc.declare_dram_parameter("recip_d_win", [128, c.EW], F32, isOutput=False)
    ids_g_d = nc.declare_dram_parameter("ids_g", [128, c.NW], F32, isOutput=False)
    ids_g1_d = nc.declare_dram_parameter("ids_g1", [128, c.NW], F32, isOutput=False)
    recip_g_d = nc.declare_dram_parameter("recip_g_win", [128, c.GW], F32, isOutput=False)
    iota_d = nc.declare_dram_parameter("iota2d", [128, 1, 128], BF16, isOutput=False)
    wparams = {}
    for name, shp in weights_shapes.items():
        wparams[name] = nc.declare_dram_parameter(name, list(shp), F32, isOutput=False)
    out_d = nc.declare_dram_parameter("out", [c.NGRAPH, c.NCLS], F32, isOutput=True)
    taps = {}
    if debug_taps:
        taps["h"] = nc.declare_dram_parameter("dbg_h", [c.NLOC, c.HID], F32, isOutput=True)
        taps["xe"] = nc.declare_dram_parameter("dbg_xe", [c.EPAD, c.HID], F32, isOutput=True)
        taps["spart"] = nc.declare_dram_parameter("dbg_spart", [c.EPAD, c.HID], F32, isOutput=True)
        taps["xT"] = nc.declare_dram_parameter("dbg_xT", [c.HID, c.NLOC], F32, isOutput=True)
        taps["yT"] = nc.declare_dram_parameter("dbg_yT", [c.HID, c.NLOC], F32, isOutput=True)

    # ---------- internal DRAM ----------
    h_dram = nc.dram_tensor("h_tab", [c.NLOC, HID], F32)
    EW_HALF = 0  # single AR (split not worth it per cost model)
    N_CC = 2 if EW_HALF > 0 else 1
    if EW_HALF > 0:
        s_part0 = nc.dram_tensor("s_part0", [EW_HALF * 128, HID], F32)
    s_part1 = nc.dram_tensor("s_part1", [c.EPAD - EW_HALF * 128, HID], F32)
    xe_dram = nc.dram_tensor("xe_tab", [c.EPAD, HID], F32, addr_space="Shared")
    xT_dram = nc.dram_tensor("xT", [HID, c.NLOC], F32)
    x0h_dram = nc.dram_tensor("x0h", [HID, c.NLOC], F32)
    yT_dram = nc.dram_tensor("yT", [HID, c.NLOC], F32)
    gsum_part = nc.dram_tensor("gsum_part", [c.GW * 128, c.NCLS], F32)
    gsum_full = nc.dram_tensor("gsum_full", [c.GW * 128, c.NCLS], F32, addr_space="Shared")

    rg = [list(range(c.NCORES))]

    with tile.TileContext(nc) as tc:
        ctx = ExitStack()
        const = ctx.enter_context(tc.tile_pool(name="const", bufs=1))
        sb = ctx.enter_context(tc.tile_pool(name="sb", bufs=2))
        gp = ctx.enter_context(tc.tile_pool(name="gp", bufs=4))
        ohp = ctx.enter_context(tc.tile_pool(name="ohp", bufs=2))
        ohgp = ctx.enter_context(tc.tile_pool(name="ohgp", bufs=2))
        flp = ctx.enter_context(tc.tile_pool(name="flp", bufs=3))
        ps_win = ctx.enter_context(tc.tile_pool(name="ps_win", bufs=2, space="PSUM"))
        ps_dense = ctx.enter_context(tc.tile_pool(name="ps_dense", bufs=2, space="PSUM"))
        ps_tr = ctx.enter_context(tc.tile_pool(name="ps_tr", bufs=1, space="PSUM"))
        ps_g = ctx.enter_context(tc.tile_pool(name="ps_g", bufs=1, space="PSUM"))

        # ---------- constants in SBUF ----------
        def load_const(dram, shape, dtype=F32):
            t = const.tile(shape, dtype, tag=f"c_{dram.name}")
            sl = tuple(slice(None) for _ in shape)
            nc.sync.dma_start(out=t[sl], in_=dram[sl])
            return t

        ident = const.tile([128, 128], F32)
        make_identity(nc, ident[:, :])
        iota = load_const(iota_d, [128, 1, 128], BF16)
        gidxA = load_const(gidxA_d, [128, sh["LA"] // 16], I16)
        idsA = load_const(idsA_d, [128, sh["nblkA"]], BF16)
        gidxB = load_const(gidxB_d, [128, sh["LB"] // 16], I16)
        idsB = load_const(idsB_d, [128, sh["nblkB"]], BF16)
        recip_c = load_const(recip_c_d, [128, c.NW])
        maskw = load_const(mask_d, [128, c.NW])
        recip_dw = load_const(recip_d_d, [128, c.EW])
        ids_g = load_const(ids_g_d, [128, c.NW])
        ids_g1 = load_const(ids_g1_d, [128, c.NW])
        recip_gw = load_const(recip_g_d, [128, c.GW])
        W = {k: load_const(v, list(v.shape)) for k, v in wparams.items()}
        # biases as [HID,1] column APs
        bias = {}
        for bn, dim in [("b_in", HID), ("b1a", HID), ("b1b", HID), ("b3", HID),
                        ("bc1", c.CLS_H)]:
            bias[bn] = W[bn]
        b2_rep = W["b2"]          # [128, HID], host-replicated
        bc2_rep = W["bc2"]        # [128, NCLS], host-replicated

        NWIN_LAST = c.NLOC - 128 * (c.NW - 1)  # rows in last node window


        def nodeblk(i):
            return slice(128 * i, min(128 * (i + 1), c.NLOC))

        def blkrows(i):
            return min(128 * (i + 1), c.NLOC) - 128 * i

        # ---------- input layer: x = relu(X @ W_in + b_in), transposed ----------
        for b in range(c.NW):
            r = blkrows(b)
            xblk = sb.tile([128, c.FT], F32, tag="xblk")
            nc.sync.dma_start(out=xblk[:r, :], in_=Xs[nodeblk(b), :])
            pt = ps_tr.tile([128, 128], F32, tag="ptr")
            nc.tensor.transpose(out=pt[:c.FT, :r], in_=xblk[:r, :c.FT], identity=ident[:r, :r])
            xTb = sb.tile([128, 128], F32, tag="xTb")
            nc.scalar.activation(out=xTb[:c.FT, :r], in_=pt[:c.FT, :r], func=AF.Copy)
            pd = ps_dense.tile([HID, 512], F32, tag="pd")
            nc.tensor.matmul(out=pd[:HID, :r], lhsT=W["W_in"][:, :], rhs=xTb[:c.FT, :r],
                             start=True, stop=True)
            xt = sb.tile([HID, 128], F32, tag="xt")
            nc.scalar.activation(out=xt[:, :r], in_=pd[:HID, :r], func=AF.Relu,
                                 bias=bias["b_in"][:, 0:1])
            nc.sync.dma_start(out=xT_dram[:, nodeblk(b)], in_=xt[:, :r])
            x0 = sb.tile([HID, 128], F32, tag="x0")
            nc.vector.tensor_scalar_mul(x0[:, :r], xt[:, :r], 0.5)
            nc.sync.dma_start(out=x0h_dram[:, nodeblk(b)], in_=x0[:, :r])

        CH = 512

        def dense_chunks():
            o = 0
            while o < c.NLOC:
                yield o, min(CH, c.NLOC - o)
                o += CH

        for layer in range(c.NLAYER):
            # ---------- h = relu(x@W1a+b1a)@W1b + b1b; write row-major table ----
            for o, n in dense_chunks():
                xt = sb.tile([HID, CH], F32, tag="xt2")
                nc.sync.dma_start(out=xt[:, :n], in_=xT_dram[:, o:o + n])
                pd = ps_dense.tile([HID, 512], F32, tag="pd")
                nc.tensor.matmul(out=pd[:HID, :n], lhsT=W["W1a"][:, :], rhs=xt[:, :n],
                                 start=True, stop=True)
                ut = sb.tile([HID, CH], F32, tag="ut")
                nc.scalar.activation(out=ut[:, :n], in_=pd[:HID, :n], func=AF.Relu,
                                     bias=bias["b1a"][:, 0:1])
                pd2 = ps_dense.tile([HID, 512], F32, tag="pd")
                nc.tensor.matmul(out=pd2[:HID, :n], lhsT=W["W1b"][:, :], rhs=ut[:, :n],
                                 start=True, stop=True)
                ht = sb.tile([HID, CH], F32, tag="ht")
                nc.vector.tensor_scalar(ht[:, :n], pd2[:HID, :n], W["b1b"][:, 0:1], None,
                                        ALU.add)
                # transpose to row-major h table
                nb = _ceil(n, 128)
                for j in range(nb):
                    r = min(128, n - 128 * j)
                    pt = ps_tr.tile([128, 128], F32, tag="ptr")
                    nc.tensor.transpose(out=pt[:r, :HID], in_=ht[:HID, 128 * j:128 * j + r],
                                        identity=ident[:HID, :HID])
                    hrm = flp.tile([128, HID], F32, tag="hrm")
                    nc.scalar.activation(out=hrm[:r, :], in_=pt[:r, :HID], func=AF.Copy)
                    nc.sync.dma_start(out=h_dram[o + 128 * j: o + 128 * j + r, :],
                                      in_=hrm[:r, :])

            # ---------- V->E: gather h[src], one-hot matmul into dst windows ----
            def make_stream(idx_tile, ids_tile, regions, dtag):
                """regions: list of (blk_start, blk_end, src_ap). Returns
                get(b) -> (g_tile, oh_tile, col) with lazy 8-block chunk
                gathers that never cross region boundaries."""
                cache = {}

                def get(b):
                    for r0, r1, src_ap in regions:
                        if r0 <= b < r1:
                            c0 = r0 + ((b - r0) // 8) * 8
                            key = c0
                            if key not in cache:
                                nb = min(8, r1 - c0)
                                gf = gp.tile([128, 8, HID], F32, tag="f" + dtag)
                                nidx = 128 * nb
                                nc.gpsimd.dma_gather(
                                    out_ap=gf[:, :nb, :], in_ap=src_ap,
                                    idxs_ap=idx_tile[:, 8 * c0: 8 * c0 + 8 * nb],
                                    num_idxs=nidx, num_idxs_reg=nidx, elem_size=HID,
                                )
                                g = gp.tile([128, 8, HID], BF16, tag=dtag)
                                nc.scalar.activation(out=g[:, :nb, :], in_=gf[:, :nb, :],
                                                     func=AF.Copy)
                                oh = ohp.tile([128, 8, 128], BF16, tag="oh" + dtag)
                                nc.vector.tensor_tensor(
                                    out=oh[:, :nb, :],
                                    in0=ids_tile[:, c0:c0 + nb].to_broadcast([128, nb, 128]),
                                    in1=iota[:, :, :].to_broadcast([128, nb, 128]),
                                    op=ALU.is_equal,
                                )
                                cache[key] = (g, oh)
                            g, oh = cache[key]
                            return g, oh, b - c0
                    raise AssertionError(b)
                return get

            offA = sh["offA"]
            BA = sh["BA"]
            getA = make_stream(gidxA, idsA, [(0, sh["nblkA"], h_dram[:, :])], "gA")
            cc_sem = nc.alloc_semaphore(f"cc{layer}")
            FB = 4  # windows per flush batch
            for w0 in range(0, c.EW, FB):
                wn = min(FB, c.EW - w0)
                sfl = flp.tile([128, FB, HID], F32, tag="sfl")
                for dw_ in range(wn):
                    w = w0 + dw_
                    nblk = int(BA[w])
                    if nblk == 0:
                        nc.vector.memset(sfl[:, dw_, :], 0.0)
                        continue
                    b0 = int(offA[w]) // 128
                    pw = ps_win.tile([128, HID], F32, tag="pw")
                    for i in range(nblk):
                        g, oh, col = getA(b0 + i)
                        nc.tensor.matmul(out=pw[:, :], lhsT=oh[:, col, :],
                                         rhs=g[:, col, :],
                                         start=(i == 0), stop=(i == nblk - 1))
                    nc.scalar.activation(
                        out=sfl[:, dw_, :], in_=pw[:, :], func=AF.Copy,
                        scale=recip_dw[:, w:w + 1])
                if w0 < EW_HALF:
                    tgt = s_part0[128 * w0:128 * (w0 + wn), :]
                else:
                    tgt = s_part1[128 * (w0 - EW_HALF):128 * (w0 - EW_HALF + wn), :]
                nc.sync.dma_start(
                    out=tgt.rearrange("(j p) c -> p j c", p=128),
                    in_=sfl[:, :wn, :])
                if w0 + wn == EW_HALF:
                    # first-half AllReduce overlaps the rest of V->E
                    with tc.tile_critical():
                        nc.gpsimd.collective_compute(
                            "AllReduce", ALU.add, replica_groups=rg,
                            ins=[s_part0.ap().opt()],
                            outs=[xe_dram[0:EW_HALF * 128, :].opt()],
                        ).then_inc(cc_sem, 1)

            # ---------- second-half AllReduce ----------
            with tc.tile_critical():
                nc.gpsimd.collective_compute(
                    "AllReduce", ALU.add, replica_groups=rg,
                    ins=[s_part1.ap().opt()],
                    outs=[xe_dram[EW_HALF * 128:c.EPAD, :].opt()],
                ).then_inc(cc_sem, 1)

            # Xe-independent dense term overlaps the AllReduce:
            # tb[m] = x@W2a + b2 per node window
            tbbuf = const.tile([128, c.NW, HID], F32, tag="tbbuf")
            for m in range(c.NW):
                if m % 4 == 0:
                    o4 = 128 * m
                    n4 = min(512, c.NLOC - o4)
                    xt4p = sb.tile([HID, 512], F32, tag="xt3")
                    nc.sync.dma_start(out=xt4p[:, :n4], in_=xT_dram[:, o4:o4 + n4])
                rows = blkrows(m)
                co = 128 * m - o4
                pdp = ps_tr.tile([128, 128], F32, tag="ptr")
                nc.tensor.matmul(out=pdp[:rows, :HID], lhsT=xt4p[:, co:co + rows],
                                 rhs=W["W2a"][:, :], start=True, stop=True)
                nc.vector.tensor_tensor(out=tbbuf[:rows, m, :], in0=pdp[:rows, :HID],
                                        in1=b2_rep[:rows, :], op=ALU.add)

            with tc.tile_critical():
                nc.gpsimd.wait_ge(cc_sem, N_CC)
            tc.strict_bb_all_engine_barrier()

            if debug_taps and layer == 0:
                nc.sync.dma_start(out=taps["h"][:, :], in_=h_dram[:, :])
                nc.sync.dma_start(out=taps["xe"][:, :], in_=xe_dram[:, :])
                nc.sync.dma_start(out=taps["spart"][:, :], in_=s_part[:, :])
            # ---------- E->V + node-window dense update ----------
            BL, BH = sh["BL"], sh["BH"]
            offL, offH = sh["offL"], sh["offH"]
            getB = make_stream(gidxB, idsB,
                               [(0, sh["nblkBL"], xe_dram[:, :]),
                                (sh["nblkBL"], sh["nblkB"], xe_dram[c.EHALF:, :])],
                               "gB")
            yt4w = {}
            for m in range(c.NW):
                if m % 4 == 0:
                    o4 = 128 * m
                    n4 = min(512, c.NLOC - o4)
                    yt4 = sb.tile([HID, 512], F32, tag="yt")
                    x04 = sb.tile([HID, 512], F32, tag="x0b")
                    nc.sync.dma_start(out=x04[:, :n4], in_=x0h_dram[:, o4:o4 + n4])
                    yt4w[m // 4] = (yt4, x04, o4, n4)
                rows = blkrows(m)
                pw = ps_win.tile([128, HID], F32, tag="pw")
                total = int(BL[m]) + int(BH[m])
                done = 0
                for nblk, off in ((int(BL[m]), int(offL[m])),
                                  (int(BH[m]), int(offH[m]))):
                    b0 = off // 128
                    for i in range(nblk):
                        g, oh, col = getB(b0 + i)
                        nc.tensor.matmul(out=pw[:, :], lhsT=oh[:, col, :],
                                         rhs=g[:, col, :],
                                         start=(done == 0),
                                         stop=(done == total - 1))
                        done += 1
                # Z window scaled by 1/deg(src)
                zw = flp.tile([128, HID], F32, tag="zw")
                if total > 0:
                    nc.scalar.activation(out=zw[:, :], in_=pw[:, :], func=AF.Copy,
                                         scale=recip_c[:, m:m + 1])
                else:
                    nc.vector.memset(zw[:, :], 0.0)
                # (Z/c) @ W2b: transpose Z window, then matmul row-major
                ptz = ps_tr.tile([128, 128], F32, tag="ptr")
                nc.tensor.transpose(out=ptz[:HID, :rows], in_=zw[:rows, :HID],
                                    identity=ident[:rows, :rows])
                zts = flp.tile([HID, 128], F32, tag="zts")
                nc.scalar.activation(out=zts[:, :rows], in_=ptz[:HID, :rows], func=AF.Copy)
                pz = ps_tr.tile([128, 128], F32, tag="pcls")
                nc.tensor.matmul(out=pz[:rows, :HID], lhsT=zts[:, :rows],
                                 rhs=W["W2b"][:, :], start=True, stop=True)
                yt4, x04, o4, n4 = yt4w[m // 4]
                co = 128 * m - o4
                xv = flp.tile([128, HID], F32, tag="xv")
                nc.vector.scalar_tensor_tensor(
                    out=xv[:rows, :], in0=tbbuf[:rows, m, :],
                    scalar=maskw[:rows, m:m + 1],
                    in1=pz[:rows, :HID], op0=ALU.mult, op1=ALU.add)
                # transpose Xv window, y = 0.5*Xv + x0h
                pt = ps_tr.tile([128, 128], F32, tag="ptr")
                nc.tensor.transpose(out=pt[:HID, :rows], in_=xv[:rows, :HID],
                                    identity=ident[:rows, :rows])
                nc.vector.scalar_tensor_tensor(
                    out=yt4[:, co:co + rows], in0=pt[:HID, :rows], scalar=0.5,
                    in1=x04[:, co:co + rows], op0=ALU.mult, op1=ALU.add)
                if m % 4 == 3 or m == c.NW - 1:
                    nc.sync.dma_start(out=yT_dram[:, o4:o4 + n4], in_=yt4[:, :n4])

            # ---------- x = relu(y @ W3 + b3) ----------
            for o, n in dense_chunks():
                yt = sb.tile([HID, CH], F32, tag="yt2")
                nc.sync.dma_start(out=yt[:, :n], in_=yT_dram[:, o:o + n])
                pd = ps_dense.tile([HID, 512], F32, tag="pd")
                nc.tensor.matmul(out=pd[:HID, :n], lhsT=W["W3"][:, :], rhs=yt[:, :n],
                                 start=True, stop=True)
                xt = sb.tile([HID, CH], F32, tag="xt4")
                nc.scalar.activation(out=xt[:, :n], in_=pd[:HID, :n], func=AF.Relu,
                                     bias=bias["b3"][:, 0:1])
                nc.sync.dma_start(out=xT_dram[:, o:o + n], in_=xt[:, :n])

        if debug_taps:
            nc.sync.dma_start(out=taps["xT"][:, :], in_=xT_dram[:, :])
            nc.sync.dma_start(out=taps["yT"][:, :], in_=yT_dram[:, :])
        # ---------- classifier + readout ----------
        gps = []
        for g in range(c.GW):
            gtile = ps_g.tile([128, c.NCLS], F32, tag=f"gps{g}")
            gps.append(gtile)
        n_mm = [0] * c.GW
        total_mm = [c.NW] * c.GW
        for o, n in dense_chunks():
            xt = sb.tile([HID, CH], F32, tag="xt5")
            nc.sync.dma_start(out=xt[:, :n], in_=xT_dram[:, o:o + n])
            pd = ps_dense.tile([HID, 512], F32, tag="pd")
            nc.tensor.matmul(out=pd[:c.CLS_H, :n], lhsT=W["Wc1"][:, :], rhs=xt[:, :n],
                             start=True, stop=True)
            ut = sb.tile([c.CLS_H, CH], F32, tag="ut2")
            nc.scalar.activation(out=ut[:, :n], in_=pd[:c.CLS_H, :n], func=AF.Relu,
                                 bias=bias["bc1"][:, 0:1])
            nb = _ceil(n, 128)
            for j in range(nb):
                b = (o + 128 * j) // 128
                r = min(128, n - 128 * j)
                pcls = ps_tr.tile([128, 128], F32, tag="pcls")
                nc.tensor.matmul(out=pcls[:r, :c.NCLS], lhsT=ut[:, 128 * j:128 * j + r],
                                 rhs=W["Wc2"][:, :], start=True, stop=True)
                cls = flp.tile([128, c.NCLS], F32, tag="cls")
                # add bc2 (replicated add via b2-style trick: bc2 [64,32]? it's [NCLS]) ->
                # bc2 is added later after readout? NO: mean of (cls+bc2) = mean(cls)+bc2.
                nc.scalar.activation(out=cls[:r, :], in_=pcls[:r, :c.NCLS], func=AF.Copy)
                for g in range(c.GW):
                    src_ids = ids_g if g == 0 else ids_g1
                    ohg = ohgp.tile([128, 128], F32, tag="ohg")
                    nc.vector.tensor_tensor(
                        out=ohg[:, :],
                        in0=src_ids[:, b:b + 1].to_broadcast([128, 128]),
                        in1=iota[:, 0, :], op=ALU.is_equal)
                    nc.tensor.matmul(out=gps[g][:, :], lhsT=ohg[:r, :],
                                     rhs=cls[:r, :],
                                     start=(n_mm[g] == 0), stop=(n_mm[g] == total_mm[g] - 1))
                    n_mm[g] += 1
        for g in range(c.GW):
            gfl = flp.tile([128, c.NCLS], F32, tag="gfl")
            nc.scalar.activation(out=gfl[:, :], in_=gps[g][:, :], func=AF.Copy)
            nc.sync.dma_start(out=gsum_part[128 * g:128 * (g + 1), :], in_=gfl[:, :])

        tc.strict_bb_all_engine_barrier()
        with tc.tile_critical():
            cc2 = nc.alloc_semaphore("cc_g")
            nc.gpsimd.collective_compute(
                "AllReduce", ALU.add, replica_groups=rg,
                ins=[gsum_part.ap().opt()], outs=[gsum_full.ap().opt()],
            ).then_inc(cc2, 1)
            nc.gpsimd.wait_ge(cc2, 1)
        tc.strict_bb_all_engine_barrier()

        # divide by counts, add bc2, write out
        for g in range(c.GW):
            gt = flp.tile([128, c.NCLS], F32, tag="gt")
            nc.sync.dma_start(out=gt[:, :], in_=gsum_full[128 * g:128 * (g + 1), :])
            go = flp.tile([128, c.NCLS], F32, tag="go")
            nc.vector.tensor_tensor(out=go[:, :], in0=gt[:, :],
                                    in1=recip_gw[:, g:g + 1].to_broadcast([128, c.NCLS]),
                                    op=ALU.mult)
            nc.vector.tensor_tensor(out=go[:, :], in0=go[:, :], in1=bc2_rep[:, :],
                                    op=ALU.add)
            rows = min(128, c.NGRAPH - 128 * g)
            nc.sync.dma_start(out=out_d[128 * g:128 * g + rows, :], in_=go[:rows, :])
        ctx.close()

    nc.finalize()
    return nc


_CACHE = {}
_LAST_RESULT = None


def _get_weights(kw, cfg):
    shapes = {
        "W_in": (cfg.FT, cfg.HID), "b_in": (cfg.HID, 1),
        "W1a": (cfg.HID, cfg.HID), "b1a": (cfg.HID, 1),
        "W1b": (cfg.HID, cfg.HID), "b1b": (cfg.HID, 1),
        "W2a": (cfg.HID, cfg.HID), "W2b": (cfg.HID, cfg.HID), "b2": (128, cfg.HID),
        "W3": (cfg.HID, cfg.HID), "b3": (cfg.HID, 1),
        "Wc1": (cfg.HID, cfg.CLS_H), "bc1": (cfg.CLS_H, 1),
        "Wc2": (cfg.CLS_H, cfg.NCLS), "bc2": (128, cfg.NCLS),
    }
    W2 = np.asarray(kw["W2"], np.float32)
    vals = {
        "W_in": kw["W_in"], "b_in": np.asarray(kw["b_in"], np.float32).reshape(-1, 1),
        "W1a": kw["W1a"], "b1a": np.asarray(kw["b1a"], np.float32).reshape(-1, 1),
        "W1b": kw["W1b"], "b1b": np.asarray(kw["b1b"], np.float32).reshape(-1, 1),
        "W2a": W2[:cfg.HID], "W2b": W2[cfg.HID:],
        "b2": np.tile(np.asarray(kw["b2"], np.float32).reshape(1, -1), (128, 1)),
        "W3": kw["W3"], "b3": np.asarray(kw["b3"], np.float32).reshape(-1, 1),
        "Wc1": kw["Wc1"], "bc1": np.asarray(kw["bc1"], np.float32).reshape(-1, 1),
        "Wc2": kw["Wc2"],
        "bc2": np.tile(np.asarray(kw["bc2"], np.float32).reshape(1, -1), (128, 1)),
    }
    vals = {k: np.ascontiguousarray(np.asarray(v, np.float32)) for k, v in vals.items()}
    return shapes, vals


def kernel(X, v2e_src, v2e_dst, all_batch, W_in, b_in, W1a, b1a, W1b, b1b,
           W2, b2, W3, b3, Wc1, bc1, Wc2, bc2, _cfg=None, _trace=False):
    cfg = _cfg or Cfg()
    kw = dict(W_in=W_in, b_in=b_in, W1a=W1a, b1a=b1a, W1b=W1b, b1b=b1b, W2=W2,
              b2=b2, W3=W3, b3=b3, Wc1=Wc1, bc1=bc1, Wc2=Wc2, bc2=bc2)
    shapes, wvals = _get_weights(kw, cfg)
    shared, in_maps = prep(cfg, np.asarray(X, np.float32), v2e_src, v2e_dst, all_batch)
    key = (cfg.N, cfg.E, tuple(shared["BA"].tolist()), tuple(shared["BL"].tolist()),
           tuple(shared["BH"].tolist()))
    if key not in _CACHE:
        _CACHE[key] = build(cfg, shared, shapes)
    nc = _CACHE[key]
    for m in in_maps:
        m.update(wvals)
    global _LAST_RESULT
    res = run_bass_kernel_spmd(nc, in_maps, core_ids=list(range(cfg.NCORES)),
                               trace=_trace)
    _LAST_RESULT = res
    return res.results[0]["out"].astype(np.float32)



# revision 30
# speedup vs baseline: 2.3244x; 2.3244x over previous
"""EquivSetGNN forward on 8 Trainium2 NeuronCores (Bass/Tile), v2.

Sharding/dataflow:
  V->E: entries partitioned by src (h table local per core), dst-sorted;
        per dst-window one-hot segment-sum matmuls into a partial Xe table
        (bf16, 128-wide padded rows); ReduceScatter gives each core the
        summed Xe rows for its 1/8 edge shard.
  E->V: entries partitioned by dst shard (Xe shard local), ordered by
        global node window; one-hot segment-sums into a partial Xv table
        over ALL nodes (core-aligned padded layout); ReduceScatter gives
        each core the summed Xv rows for its local nodes.
  Dense MLP phases run feature-transposed [64, n] with x/x0/tbbuf resident
  in SBUF (bf16); all dense matmuls bf16 (1 cycle/row).
  Readout: per-graph one-hot matmul sums + small AllReduce + 1/count.
"""
import sys

sys.path.insert(0, "/opt/trn_rl_repo")

import ml_dtypes
import numpy as np

import concourse.bass as bass
import concourse.bacc as bacc
import concourse.mybir as mybir
import concourse.tile as tile
from concourse.bass_utils import run_bass_kernel_spmd
from concourse.masks import make_identity
from contextlib import ExitStack

F32 = mybir.dt.float32
BF16 = mybir.dt.bfloat16
I16 = mybir.dt.int16
U32 = mybir.dt.uint32
AF = mybir.ActivationFunctionType
ALU = mybir.AluOpType
BF = ml_dtypes.bfloat16


class Cfg:
    def __init__(self, N=100000, E=50000, FT=128, HID=64, CLS_H=64, NCLS=32,
                 NGRAPH=256, NLAYER=2, NCORES=8):
        self.N, self.E, self.FT, self.HID = N, E, FT, HID
        self.CLS_H, self.NCLS, self.NGRAPH, self.NLAYER = CLS_H, NCLS, NGRAPH, NLAYER
        self.NCORES = NCORES
        assert N % NCORES == 0
        self.NLOC = N // NCORES              # 12500 local nodes
        self.NW = -(-self.NLOC // 128)       # 98 local node windows
        self.EW = -(-E // 128)               # 391 edge windows
        self.EPAD = self.EW * 128            # 50048
        assert self.EPAD % NCORES == 0
        self.ESH = self.EPAD // NCORES       # 6256 edge shard rows
        self.EWL = 256                       # V->E windows in first RS chunk
        self.ESHL = self.EWL * 128 // NCORES          # 3072
        self.ESHH = (self.EPAD - self.EWL * 128) // NCORES  # 3184
        self.NSH = 128 * (-(-self.NLOC // 128) * 128 // 128)  # padded local rows
        # pad local node count to a value s.t. 8*NSH is window-divisible
        self.NSH = -(-self.NLOC // 16) * 16  # 12512 (so NPAD=100096=782*128)
        self.NPAD = self.NSH * NCORES        # 100096
        self.NWG = self.NPAD // 128          # 782 global node windows
        assert self.NPAD % 128 == 0
        self.GW = -(-NGRAPH // 128)          # 2 graph windows


def _ceil(a, b):
    return -(-a // b)


def _wrap16(idx):
    """flat idx array -> [128, L/16] int16 wrapped layout (replicated x8)."""
    a = np.asarray(idx, np.int16).reshape(-1, 16).T
    return np.ascontiguousarray(np.tile(a, (8, 1)))


def _gath_layout(vals, dtype):
    """flat [L] -> [128, 1, L/128] layout (entry k at [k%128, 0, k//128])."""
    L = len(vals)
    assert L % 128 == 0
    return np.ascontiguousarray(
        np.asarray(vals, dtype).reshape(L // 128, 128).T.reshape(128, 1, L // 128))


def prep(cfg, X, v2e_src, v2e_dst, all_batch):
    """Host preprocessing -> (shared_structure, per_core_input_maps)."""
    c = cfg
    src = np.asarray(v2e_src, np.int64)
    dst = np.asarray(v2e_dst, np.int64)
    batch = np.asarray(all_batch, np.int64)

    d_deg = np.bincount(dst, minlength=c.E).astype(np.float32)
    c_deg = np.bincount(src, minlength=c.N).astype(np.float32)
    recip_d = (1.0 / np.maximum(d_deg, 1.0)).astype(np.float32)
    recip_c = (1.0 / np.maximum(c_deg, 1.0)).astype(np.float32)

    # ---- V->E stream: src-partitioned, dst-sorted ----
    cntA = np.zeros((c.NCORES, c.EW), np.int64)
    coreA = []
    for ci in range(c.NCORES):
        lo, hi = np.searchsorted(src, [c.NLOC * ci, c.NLOC * (ci + 1)])
        s = (src[lo:hi] - c.NLOC * ci).astype(np.int64)
        e = dst[lo:hi]
        order = np.argsort(e, kind="stable")
        sA, eA = s[order], e[order]
        win = eA >> 7
        cntA[ci] = np.bincount(win, minlength=c.EW)
        coreA.append((sA, eA, win))
    BA = -(-cntA.max(axis=0) // 128)
    capA = BA * 128
    offA = np.concatenate([[0], np.cumsum(capA)])
    LA = int(offA[-1])
    nblkA = LA // 128

    # ---- E->V stream: dst-shard partitioned, global-node-window ordered ----
    cntB = np.zeros((c.NCORES, c.NWG), np.int64)
    coreB = []
    owner = dst // c.ESH
    local_e = dst - c.ESH * owner
    for ci in range(c.NCORES):
        m = owner == ci
        sg = src[m]
        el = local_e[m]
        r = c.NSH * (sg // c.NLOC) + (sg % c.NLOC)
        win = r >> 7
        order = np.argsort(win, kind="stable")
        rB, eB, winB = r[order], el[order], win[order]
        cntB[ci] = np.bincount(winB, minlength=c.NWG)
        coreB.append((rB, eB, winB))
    BB = -(-cntB.max(axis=0) // 128)
    capB = BB * 128
    offB = np.concatenate([[0], np.cumsum(capB)])
    LB = int(offB[-1])
    nblkB = LB // 128

    shared = dict(BA=BA, BB=BB, LA=LA, LB=LB, nblkA=nblkA, nblkB=nblkB,
                  offA=offA, offB=offB)

    # graph readout constants
    gcnt = np.bincount(batch, minlength=c.NGRAPH).astype(np.float32)
    recip_g = (1.0 / np.maximum(gcnt, 1.0)).astype(np.float32)
    recip_g_win = np.zeros((128, c.GW), np.float32)
    for g in range(c.NGRAPH):
        recip_g_win[g % 128, g // 128] = recip_g[g]

    iota_e = np.broadcast_to(
        np.arange(128, dtype=BF)[None, :, None], (128, 128, 16))
    iota_e = np.ascontiguousarray(iota_e)

    in_maps = []
    for ci in range(c.NCORES):
        # V->E placement
        sA, eA, winA = coreA[ci]
        startsA = np.searchsorted(winA, np.arange(c.EW))
        place = offA[winA] + (np.arange(len(winA)) - startsA[winA])
        gidxA = np.zeros(LA, np.int64)
        idsA = np.full(LA, -1.0, np.float32)
        gidxA[place] = sA
        idsA[place] = (eA - (winA << 7)).astype(np.float32)

        # E->V placement
        rB, eB, winB = coreB[ci]
        startsB = np.searchsorted(winB, np.arange(c.NWG))
        place = offB[winB] + (np.arange(len(winB)) - startsB[winB])
        gidxB = np.zeros(LB, np.int64)
        idsB = np.full(LB, -1.0, np.float32)
        gidxB[place] = eB
        idsB[place] = (rB & 127).astype(np.float32)

        # per-local-node arrays in window layout [128, NW]
        npad = c.NW * 128
        cw = np.zeros(npad, np.float32)
        cw[:c.NLOC] = recip_c[c.NLOC * ci: c.NLOC * (ci + 1)]
        recip_c_win = np.ascontiguousarray(cw.reshape(c.NW, 128).T)
        mw = np.zeros(npad, np.float32)
        mw[:c.NLOC] = (c_deg[c.NLOC * ci: c.NLOC * (ci + 1)] > 0).astype(np.float32)
        mask_win = np.ascontiguousarray(mw.reshape(c.NW, 128).T)
        bw = np.full(npad, -1.0, np.float32)
        bw[:c.NLOC] = batch[c.NLOC * ci: c.NLOC * (ci + 1)].astype(np.float32)
        ids_g = np.ascontiguousarray(
            bw.reshape(c.NW, 128).T.astype(BF).reshape(128, 1, c.NW))
        bw1 = np.where(bw < 0, -1.0, bw - 128.0).astype(np.float32)
        ids_g1 = np.ascontiguousarray(
            bw1.reshape(c.NW, 128).T.astype(BF).reshape(128, 1, c.NW))

        dw = np.zeros(c.EW * 128, np.float32)
        dw[:c.E] = recip_d
        recip_d_win = np.ascontiguousarray(dw.reshape(c.EW, 128).T)

        m = {
            "Xs": np.ascontiguousarray(X[c.NLOC * ci: c.NLOC * (ci + 1)]).astype(np.float32),
            "gidxA": _wrap16(gidxA), "idsA": _gath_layout(idsA, BF),
            "gidxB": _wrap16(gidxB), "idsB": _gath_layout(idsB, BF),
            "recip_c_win": recip_c_win,
            "recip_d_win": recip_d_win, "ids_g": ids_g, "ids_g1": ids_g1,
            "recip_g_win": recip_g_win, "iota_e": iota_e,
        }
        in_maps.append(m)
    return shared, in_maps


def build(cfg, sh, debug=False):
    """Build the SPMD Bass program."""
    c = cfg
    nc = bacc.Bacc("TRN2", debug=False, num_swdge_queues=1)
    HID = c.HID
    nblkA, nblkB = sh["nblkA"], sh["nblkB"]

    # ---------- I/O ----------
    Xs = nc.declare_dram_parameter("Xs", [c.NLOC, c.FT], F32, isOutput=False)
    gidxA_d = nc.declare_dram_parameter("gidxA", [128, sh["LA"] // 16], I16, isOutput=False)
    idsA_d = nc.declare_dram_parameter("idsA", [128, 1, nblkA], BF16, isOutput=False)
    gidxB_d = nc.declare_dram_parameter("gidxB", [128, sh["LB"] // 16], I16, isOutput=False)
    idsB_d = nc.declare_dram_parameter("idsB", [128, 1, nblkB], BF16, isOutput=False)
    recip_c_d = nc.declare_dram_parameter("recip_c_win", [128, c.NW], F32, isOutput=False)
    recip_d_d = nc.declare_dram_parameter("recip_d_win", [128, c.EW], F32, isOutput=False)
    ids_g_d = nc.declare_dram_parameter("ids_g", [128, 1, c.NW], BF16, isOutput=False)
    ids_g1_d = nc.declare_dram_parameter("ids_g1", [128, 1, c.NW], BF16, isOutput=False)
    recip_g_d = nc.declare_dram_parameter("recip_g_win", [128, c.GW], F32, isOutput=False)
    iota_d = nc.declare_dram_parameter("iota_e", [128, 128, 16], BF16, isOutput=False)
    wdecl = {
        "W_in": ([c.FT, HID], BF16), "W1a": ([HID, HID], BF16),
        "W1b": ([HID, HID], BF16), "W2a": ([HID, HID], BF16),
        "W2b": ([HID, HID], BF16), "W3": ([HID, HID], BF16),
        "Wc1": ([HID, c.CLS_H], BF16), "Wc2": ([c.CLS_H, c.NCLS], BF16),
        "b_in": ([HID, 1], F32), "b1a": ([HID, 1], F32), "b1b": ([HID, 1], F32),
        "b3": ([HID, 1], F32), "bc1": ([c.CLS_H, 1], F32),
        "bc2_rep": ([128, c.NCLS], F32), "b2h": ([HID, 1], F32),
    }
    wparams = {k: nc.declare_dram_parameter(k, list(s), d, isOutput=False)
               for k, (s, d) in wdecl.items()}
    out_d = nc.declare_dram_parameter("out", [c.NGRAPH, c.NCLS], F32, isOutput=True)
    taps = {}
    if debug:
        taps["h"] = nc.declare_dram_parameter("dbg_h", [c.NW * 128, 128], BF16, isOutput=True)
        taps["xe"] = nc.declare_dram_parameter("dbg_xe", [c.ESH, 128], BF16, isOutput=True)
        taps["zv"] = nc.declare_dram_parameter("dbg_zv", [c.NSH, c.HID], BF16, isOutput=True)
        taps["xT"] = nc.declare_dram_parameter("dbg_xT", [c.HID, c.NLOC], BF16, isOutput=True)
        taps["tbbh"] = nc.declare_dram_parameter("dbg_tbbh", [c.HID, c.NLOC], BF16, isOutput=True)
        taps["xT1"] = nc.declare_dram_parameter("dbg_xT1", [c.HID, c.NLOC], BF16, isOutput=True)
        taps["zv1"] = nc.declare_dram_parameter("dbg_zv1", [c.NSH, c.HID], BF16, isOutput=True)
        taps["xe1"] = nc.declare_dram_parameter("dbg_xe1", [c.ESH, 128], BF16, isOutput=True)

    # ---------- internal DRAM ----------
    h_dram = nc.dram_tensor("h_tab", [c.NW * 128, 128], BF16)
    s_part = nc.dram_tensor("s_part", [c.EPAD, 128], BF16)
    xe_sh = nc.dram_tensor("xe_sh", [c.ESH, 128], BF16)
    xv_part = nc.dram_tensor("xv_part", [c.NPAD, HID], BF16)
    zv_sh = nc.dram_tensor("zv_sh", [c.NSH, HID], BF16)
    gsum_part = nc.dram_tensor("gsum_part", [c.GW * 128, c.NCLS], F32)
    gsum_full = nc.dram_tensor("gsum_full", [c.GW * 128, c.NCLS], F32,
                               addr_space="Shared")

    rg = [list(range(c.NCORES))]

    with tile.TileContext(nc) as tc:
        ctx = ExitStack()
        const = ctx.enter_context(tc.tile_pool(name="const", bufs=1))
        sb = ctx.enter_context(tc.tile_pool(name="sb", bufs=3))
        gp = ctx.enter_context(tc.tile_pool(name="gp", bufs=4))
        ohp = ctx.enter_context(tc.tile_pool(name="ohp", bufs=3))
        flp = ctx.enter_context(tc.tile_pool(name="flp", bufs=3))
        ps_big = ctx.enter_context(tc.tile_pool(name="ps_big", bufs=2, space="PSUM"))
        ps_dense = ctx.enter_context(tc.tile_pool(name="ps_dense", bufs=2, space="PSUM"))
        ps_g = ctx.enter_context(tc.tile_pool(name="ps_g", bufs=1, space="PSUM"))

        def load_const(dram, shape, dtype=F32):
            t = const.tile(shape, dtype, tag=f"c_{dram.name}")
            sl = tuple(slice(None) for _ in shape)
            nc.sync.dma_start(out=t[sl], in_=dram[sl])
            return t

        ident = const.tile([128, 128], F32)
        make_identity(nc, ident[:, :])
        ident_bf = const.tile([128, 128], BF16)
        nc.scalar.activation(out=ident_bf[:, :], in_=ident[:, :], func=AF.Copy)
        W = {k: load_const(v, wdecl[k][0], wdecl[k][1]) for k, v in wparams.items()}

        # resident activations
        xT = const.tile([HID, c.NLOC], BF16, tag="xT")
        x0h = const.tile([HID, c.NLOC], BF16, tag="x0h")
        tbbh = const.tile([HID, c.NLOC], BF16, tag="tbbh")

        CH = 512

        def chunks():
            o = 0
            while o < c.NLOC:
                yield o, min(CH, c.NLOC - o)
                o += CH

        def blkrows(m):
            return min(128, c.NLOC - 128 * m)

        # ---------- input layer: xT = relu(W_in^T @ X^T), x0h = 0.5 xT ------
        for o, n in chunks():
            xblk = sb.tile([128, 4, c.FT], F32, tag="xblk")
            if n % 128 == 0:
                nc.sync.dma_start(
                    out=xblk[:, :n // 128, :],
                    in_=Xs[o:o + n, :].rearrange("(j p) c -> p j c", p=128))
            else:
                for j in range(_ceil(n, 128)):
                    r = min(128, n - 128 * j)
                    nc.sync.dma_start(out=xblk[:r, j, :],
                                      in_=Xs[o + 128 * j:o + 128 * j + r, :])
            ptx = ps_big.tile([128, 512], F32, tag="pbig")
            for j in range(_ceil(n, 128)):
                r = min(128, n - 128 * j)
                nc.tensor.transpose(out=ptx[:c.FT, 128 * j:128 * j + r],
                                    in_=xblk[:r, j, :], identity=ident[:r, :r])
            xTb = sb.tile([128, 512], BF16, tag="xTb")
            nc.vector.tensor_scalar_mul(xTb[:, :n], ptx[:, :n], 1.0)
            pd = ps_dense.tile([HID, 512], F32, tag="pd")
            nc.tensor.matmul(out=pd[:, :n], lhsT=W["W_in"][:, :], rhs=xTb[:, :n],
                             start=True, stop=True)
            nc.scalar.activation(out=xT[:, o:o + n], in_=pd[:, :n], func=AF.Relu,
                                 bias=W["b_in"][:, 0:1])
            nc.vector.tensor_scalar_mul(x0h[:, o:o + n], xT[:, o:o + n], 0.5)

        iota_e = load_const(iota_d, [128, 128, 16], BF16)
        gidxA = load_const(gidxA_d, [128, sh["LA"] // 16], I16)
        idsA = load_const(idsA_d, [128, 1, nblkA], BF16)
        gidxB = load_const(gidxB_d, [128, sh["LB"] // 16], I16)
        idsB = load_const(idsB_d, [128, 1, nblkB], BF16)
        recip_c = load_const(recip_c_d, [128, c.NW])
        recip_dw = load_const(recip_d_d, [128, c.EW])
        ids_g = load_const(ids_g_d, [128, 1, c.NW], BF16)
        ids_g1 = load_const(ids_g1_d, [128, 1, c.NW], BF16)
        recip_gw = load_const(recip_g_d, [128, c.GW])

        BA, BB = sh["BA"], sh["BB"]
        offA, offB = sh["offA"], sh["offB"]

        def make_stream(idx_tile, ids_tile, src_ap, nblk_total, dtag):
            cache = {}

            def get(b):
                c0 = (b // 8) * 8
                if c0 not in cache:
                    nb = min(8, nblk_total - c0)
                    g = gp.tile([128, 8, 64], U32, tag=dtag)
                    nc.gpsimd.dma_gather(
                        out_ap=g[:, :nb, :], in_ap=src_ap.bitcast(U32),
                        idxs_ap=idx_tile[:, 8 * c0: 8 * c0 + 8 * nb],
                        num_idxs=128 * nb, num_idxs_reg=128 * nb, elem_size=64)
                    oh = ohp.tile([128, 128, 8], BF16, tag="oh" + dtag)
                    nc.vector.tensor_tensor(
                        out=oh[:, :, :nb],
                        in0=ids_tile[:, 0:1, c0:c0 + nb].to_broadcast([128, 128, nb]),
                        in1=iota_e[:, :, :nb], op=ALU.is_equal)
                    cache[c0] = (g, oh)
                g, oh = cache[c0]
                return g, oh, b - c0
            return get

        for layer in range(c.NLAYER):
            # ---------- h = relu(x@W1a+b1a)@W1b + b1b -> bf16 table ----------
            ps_h_cm = tc.tile_pool(name=f"ps_h{layer}", bufs=2, space="PSUM")
            ps_h = ps_h_cm.__enter__()
            for o, n in chunks():
                pd = ps_dense.tile([HID, 512], F32, tag="pd")
                nc.tensor.matmul(out=pd[:, :n], lhsT=W["W1a"][:, :], rhs=xT[:, o:o + n],
                                 start=True, stop=True)
                ut = sb.tile([HID, 512], BF16, tag="ut")
                nc.scalar.activation(out=ut[:, :n], in_=pd[:, :n], func=AF.Relu,
                                     bias=W["b1a"][:, 0:1])
                pd2 = ps_dense.tile([HID, 512], F32, tag="pd")
                nc.tensor.matmul(out=pd2[:, :n], lhsT=W["W1b"][:, :], rhs=ut[:, :n],
                                 start=True, stop=True)
                htb = sb.tile([HID, 512], BF16, tag="htb")
                nc.vector.tensor_scalar(htb[:, :n], pd2[:, :n], W["b1b"][:, 0:1],
                                        None, ALU.add)
                nb = _ceil(n, 128)
                ptb = ps_h.tile([128, 4, HID], BF16, tag="ptb")
                for j in range(nb):
                    nc.tensor.transpose(out=ptb[:, j, :],
                                        in_=htb[:, 128 * j:128 * (j + 1)],
                                        identity=ident_bf[:HID, :HID])
                hrm = flp.tile([128, 4, HID], BF16, tag="hrm")
                nc.scalar.activation(out=hrm[:, :nb, :],
                                     in_=ptb[:, :nb, :], func=AF.Copy)
                nc.sync.dma_start(
                    out=h_dram[o:o + 128 * nb, 0:HID].rearrange(
                        "(j p) c -> p j c", p=128),
                    in_=hrm[:, :nb, :])
            ps_h_cm.__exit__(None, None, None)

            if debug and layer == 0:
                nc.sync.dma_start(out=taps["h"][:, :], in_=h_dram[:, :])
            # ---------- V->E partials ----------
            ps_ve_cm = tc.tile_pool(name=f"ps_ve{layer}", bufs=2, space="PSUM")
            ps_ve = ps_ve_cm.__enter__()
            getA = make_stream(gidxA, idsA, h_dram[:, :], nblkA, "gA")
            cc1 = nc.alloc_semaphore(f"cc_xe{layer}")
            for w0 in range(0, c.EW, 8):
                wn = min(8, c.EW - w0)
                sfl = flp.tile([128, 8, HID], BF16, tag="sflA")
                for dw_ in range(wn):
                    w = w0 + dw_
                    nblk = int(BA[w])
                    if nblk == 0:
                        nc.vector.memset(sfl[:, dw_, :], 0.0)
                        continue
                    b0 = int(offA[w]) // 128
                    pw = ps_ve.tile([128, HID], F32, tag="pw")
                    for i in range(nblk):
                        g, oh, col = getA(b0 + i)
                        nc.tensor.matmul(out=pw[:, :], lhsT=oh[:, :, col],
                                         rhs=g[:, col, 0:HID // 2].bitcast(BF16),
                                         start=(i == 0), stop=(i == nblk - 1))
                    nc.scalar.activation(out=sfl[:, dw_, :], in_=pw[:, :],
                                         func=AF.Copy, scale=recip_dw[:, w:w + 1])
                nc.sync.dma_start(
                    out=s_part[128 * w0:128 * (w0 + wn), 0:HID].rearrange(
                        "(j p) c -> p j c", p=128),
                    in_=sfl[:, :wn, :])
            ps_ve_cm.__exit__(None, None, None)

            # ---------- ReduceScatter Xe ----------
            with tc.tile_critical():
                nc.gpsimd.collective_compute(
                    "ReduceScatter", ALU.add, replica_groups=rg,
                    ins=[s_part.ap().opt()], outs=[xe_sh.ap().opt()],
                ).then_inc(cc1, 1)

            # overlap: tbbh = 0.5*(x@W2a + b2) + x0h   (transposed space)
            for o, n in list(chunks())[:13]:
                pdp = ps_dense.tile([HID, 512], F32, tag="pd")
                nc.tensor.matmul(out=pdp[:, :n], lhsT=W["W2a"][:, :],
                                 rhs=xT[:, o:o + n], start=True, stop=True)
                nc.vector.scalar_tensor_tensor(
                    out=tbbh[:, o:o + n], in0=pdp[:, :n],
                    scalar=W["b2h"][:, 0:1], in1=x0h[:, o:o + n],
                    op0=ALU.add, op1=ALU.add)

            with tc.tile_critical():
                nc.gpsimd.wait_ge(cc1, 1)
            tc.strict_bb_all_engine_barrier()

            if debug:
                nc.sync.dma_start(out=taps["xe" if layer == 0 else "xe1"][:, :],
                                  in_=xe_sh[:, :])
            # ---------- E->V partials over global padded node windows -------
            getB = make_stream(gidxB, idsB, xe_sh[:, :], nblkB, "gB")
            for w0 in range(0, c.NWG, 8):
                wn = min(8, c.NWG - w0)
                pwf = ps_big.tile([128, 512], F32, tag="pbig")
                for dw_ in range(wn):
                    w = w0 + dw_
                    nblk = int(BB[w])
                    if nblk == 0:
                        nc.vector.memset(pwf[:, HID * dw_:HID * (dw_ + 1)], 0.0)
                        continue
                    b0 = int(offB[w]) // 128
                    for i in range(nblk):
                        g, oh, col = getB(b0 + i)
                        nc.tensor.matmul(out=pwf[:, HID * dw_:HID * (dw_ + 1)],
                                         lhsT=oh[:, :, col],
                                         rhs=g[:, col, 0:HID // 2].bitcast(BF16),
                                         start=(i == 0), stop=(i == nblk - 1))
                sfl2 = flp.tile([128, 8, HID], BF16, tag="sflB")
                nc.scalar.activation(
                    out=sfl2[:, :wn, :],
                    in_=pwf[:, :HID * wn].rearrange("p (j c) -> p j c", c=HID),
                    func=AF.Copy)
                nc.sync.dma_start(
                    out=xv_part[128 * w0:128 * (w0 + wn), :].rearrange(
                        "(j p) c -> p j c", p=128),
                    in_=sfl2[:, :wn, :])

            # ---------- ReduceScatter Xv ----------
            cc2 = nc.alloc_semaphore(f"cc_xv{layer}")
            with tc.tile_critical():
                nc.gpsimd.collective_compute(
                    "ReduceScatter", ALU.add, replica_groups=rg,
                    ins=[xv_part.ap().opt()], outs=[zv_sh.ap().opt()],
                ).then_inc(cc2, 1)
            for o, n in list(chunks())[13:]:
                pdp = ps_dense.tile([HID, 512], F32, tag="pd")
                nc.tensor.matmul(out=pdp[:, :n], lhsT=W["W2a"][:, :],
                                 rhs=xT[:, o:o + n], start=True, stop=True)
                nc.vector.scalar_tensor_tensor(
                    out=tbbh[:, o:o + n], in0=pdp[:, :n],
                    scalar=W["b2h"][:, 0:1], in1=x0h[:, o:o + n],
                    op0=ALU.add, op1=ALU.add)
            with tc.tile_critical():
                nc.gpsimd.wait_ge(cc2, 1)
            tc.strict_bb_all_engine_barrier()

            if debug:
                nc.sync.dma_start(out=taps["zv" if layer == 0 else "zv1"][:, :],
                                  in_=zv_sh[:, :])
                tbd = flp.tile([HID, 512], BF16, tag="tbd")
                for o, n in chunks():
                    nc.vector.tensor_scalar_mul(tbd[:, :n], tbbh[:, o:o + n], 1.0)
                    nc.sync.dma_start(out=taps["tbbh"][:, o:o + n], in_=tbd[:, :n])
            # ---------- dense update: x = relu((tbbh + (z*recip)@W2b_h) @ W3 + b3)
            ps_up_cm = tc.tile_pool(name=f"ps_up{layer}", bufs=2, space="PSUM")
            ps_up = ps_up_cm.__enter__()
            yt = None
            zv4 = None
            pzg = None
            for m in range(c.NW):
                rows = blkrows(m)
                if m % 4 == 0:
                    o4 = 128 * m
                    n4 = min(512, c.NLOC - o4)
                    yt = sb.tile([HID, 512], BF16, tag="yt")
                    pzg = ps_dense.tile([HID, 512], F32, tag="pd")
                    zv4 = sb.tile([128, 4, HID], BF16, tag="zv4")
                    if o4 + 512 <= c.NSH:
                        nc.sync.dma_start(
                            out=zv4[:, :, :],
                            in_=zv_sh[o4:o4 + 512, :].rearrange(
                                "(j p) c -> p j c", p=128))
                    else:
                        for j in range(_ceil(c.NSH - o4, 128)):
                            zr = min(128, c.NSH - o4 - 128 * j)
                            nc.sync.dma_start(
                                out=zv4[:zr, j, :],
                                in_=zv_sh[o4 + 128 * j:o4 + 128 * j + zr, :])
                co = 128 * m - o4
                zs = flp.tile([128, HID], BF16, tag="zs")
                nc.vector.tensor_scalar(zs[:rows, :], zv4[:rows, m % 4, :],
                                        recip_c[:rows, m:m + 1], None, ALU.mult)
                ptz = ps_up.tile([HID, 128], BF16, tag="ptz")
                nc.tensor.transpose(out=ptz[:, :rows], in_=zs[:rows, :],
                                    identity=ident_bf[:rows, :rows])
                zts = sb.tile([HID, 128], BF16, tag="zts")
                nc.scalar.activation(out=zts[:, :rows], in_=ptz[:, :rows], func=AF.Copy)
                nc.tensor.matmul(out=pzg[:, co:co + rows], lhsT=W["W2b"][:, :],
                                 rhs=zts[:, :rows], start=True, stop=True)
                if m % 4 == 3 or m == c.NW - 1:
                    nc.vector.tensor_tensor(out=yt[:, :n4], in0=pzg[:, :n4],
                                            in1=tbbh[:, o4:o4 + n4], op=ALU.add)
                    pd3 = ps_dense.tile([HID, 512], F32, tag="pd")
                    nc.tensor.matmul(out=pd3[:, :n4], lhsT=W["W3"][:, :],
                                     rhs=yt[:, :n4], start=True, stop=True)
                    nc.scalar.activation(out=xT[:, o4:o4 + n4], in_=pd3[:, :n4],
                                         func=AF.Relu, bias=W["b3"][:, 0:1])
            ps_up_cm.__exit__(None, None, None)
            if debug and layer == 0:
                xtd0 = flp.tile([HID, 512], BF16, tag="xtd0")
                for o, n in chunks():
                    nc.vector.tensor_scalar_mul(xtd0[:, :n], xT[:, o:o + n], 1.0)
                    nc.sync.dma_start(out=taps["xT"][:, o:o + n], in_=xtd0[:, :n])

        if debug:
            xtd1 = flp.tile([HID, 512], BF16, tag="xtd1")
            for o, n in chunks():
                nc.vector.tensor_scalar_mul(xtd1[:, :n], xT[:, o:o + n], 1.0)
                nc.sync.dma_start(out=taps["xT1"][:, o:o + n], in_=xtd1[:, :n])
        # ---------- classifier + readout ----------
        ps_cl_cm = tc.tile_pool(name="ps_cl", bufs=2, space="PSUM")
        ps_cl = ps_cl_cm.__enter__()
        gps = []
        for g in range(c.GW):
            gtile = ps_g.tile([128, c.NCLS], F32, tag=f"gps{g}")
            gps.append(gtile)
        n_mm = [0, 0]
        for o, n in chunks():
            pd = ps_dense.tile([HID, 512], F32, tag="pd")
            nc.tensor.matmul(out=pd[:c.CLS_H, :n], lhsT=W["Wc1"][:, :],
                             rhs=xT[:, o:o + n], start=True, stop=True)
            ut = sb.tile([c.CLS_H, 512], BF16, tag="utc")
            nc.scalar.activation(out=ut[:, :n], in_=pd[:c.CLS_H, :n], func=AF.Relu,
                                 bias=W["bc1"][:, 0:1])
            nb = _ceil(n, 128)
            b0 = o // 128
            ohgs = []
            for gi, idst in ((0, ids_g), (1, ids_g1)):
                ohg = ohp.tile([128, 128, 4], BF16, tag=f"ohg{gi}")
                nc.vector.tensor_tensor(
                    out=ohg[:, :, :nb],
                    in0=idst[:, 0:1, b0:b0 + nb].to_broadcast([128, 128, nb]),
                    in1=iota_e[:, :, :nb], op=ALU.is_equal)
                ohgs.append(ohg)
            for j in range(nb):
                r = min(128, n - 128 * j)
                pcls = ps_cl.tile([128, HID], F32, tag="pcls")
                nc.tensor.matmul(out=pcls[:r, :c.NCLS], lhsT=ut[:, 128 * j:128 * j + r],
                                 rhs=W["Wc2"][:, :], start=True, stop=True)
                cls = flp.tile([128, c.NCLS], BF16, tag="cls")
                nc.scalar.activation(out=cls[:r, :], in_=pcls[:r, :c.NCLS], func=AF.Copy)
                for gi in range(c.GW):
                    nc.tensor.matmul(out=gps[gi][:, :], lhsT=ohgs[gi][:r, :, j],
                                     rhs=cls[:r, :],
                                     start=(n_mm[gi] == 0), stop=(n_mm[gi] == c.NW - 1))
                    n_mm[gi] += 1
        for g in range(c.GW):
            gfl = flp.tile([128, c.NCLS], F32, tag="gfl")
            nc.scalar.activation(out=gfl[:, :], in_=gps[g][:, :], func=AF.Copy)
            nc.sync.dma_start(out=gsum_part[128 * g:128 * (g + 1), :], in_=gfl[:, :])
        ps_cl_cm.__exit__(None, None, None)

        tc.strict_bb_all_engine_barrier()
        with tc.tile_critical():
            cc3 = nc.alloc_semaphore("cc_g")
            nc.gpsimd.collective_compute(
                "AllReduce", ALU.add, replica_groups=rg,
                ins=[gsum_part.ap().opt()], outs=[gsum_full.ap().opt()],
            ).then_inc(cc3, 1)
            nc.gpsimd.wait_ge(cc3, 1)
        tc.strict_bb_all_engine_barrier()

        for g in range(c.GW):
            gt = flp.tile([128, c.NCLS], F32, tag="gt")
            nc.sync.dma_start(out=gt[:, :], in_=gsum_full[128 * g:128 * (g + 1), :])
            go = flp.tile([128, c.NCLS], F32, tag="go")
            nc.vector.tensor_tensor(
                out=go[:, :], in0=gt[:, :],
                in1=recip_gw[:, g:g + 1].to_broadcast([128, c.NCLS]), op=ALU.mult)
            nc.vector.tensor_tensor(out=go[:, :], in0=go[:, :],
                                    in1=W["bc2_rep"][:, :], op=ALU.add)
            rows = min(128, c.NGRAPH - 128 * g)
            nc.sync.dma_start(out=out_d[128 * g:128 * g + rows, :], in_=go[:rows, :])
        ctx.close()

    nc.finalize()
    return nc


_CACHE = {}
_LAST_RESULT = None


def _get_weights(kw, cfg):
    """Host weight conversion; W2 split into 0.5*W2a/0.5*W2b; b2h = 0.5*b2."""
    W2 = np.asarray(kw["W2"], np.float32)
    vals = {
        "W_in": np.asarray(kw["W_in"], np.float32).astype(BF),
        "W1a": np.asarray(kw["W1a"], np.float32).astype(BF),
        "W1b": np.asarray(kw["W1b"], np.float32).astype(BF),
        "W2a": (0.5 * W2[:cfg.HID]).astype(BF), "W2b": (0.5 * W2[cfg.HID:]).astype(BF),
        "W3": np.asarray(kw["W3"], np.float32).astype(BF),
        "Wc1": np.asarray(kw["Wc1"], np.float32).astype(BF),
        "Wc2": np.asarray(kw["Wc2"], np.float32).astype(BF),
        "b_in": np.asarray(kw["b_in"], np.float32).reshape(-1, 1),
        "b1a": np.asarray(kw["b1a"], np.float32).reshape(-1, 1),
        "b1b": np.asarray(kw["b1b"], np.float32).reshape(-1, 1),
        "b3": np.asarray(kw["b3"], np.float32).reshape(-1, 1),
        "bc1": np.asarray(kw["bc1"], np.float32).reshape(-1, 1),
        "bc2_rep": np.tile(np.asarray(kw["bc2"], np.float32).reshape(1, -1), (128, 1)),
        "b2h": 0.5 * np.asarray(kw["b2"], np.float32).reshape(-1, 1),
    }
    return {k: np.ascontiguousarray(v) for k, v in vals.items()}


def kernel(X, v2e_src, v2e_dst, all_batch, W_in, b_in, W1a, b1a, W1b, b1b,
           W2, b2, W3, b3, Wc1, bc1, Wc2, bc2, _cfg=None, _trace=False,
           _debug=False):
    cfg = _cfg or Cfg()
    kw = dict(W_in=W_in, b_in=b_in, W1a=W1a, b1a=b1a, W1b=W1b, b1b=b1b, W2=W2,
              b2=b2, W3=W3, b3=b3, Wc1=Wc1, bc1=bc1, Wc2=Wc2, bc2=bc2)
    shared, in_maps = prep(cfg, np.asarray(X, np.float32), v2e_src, v2e_dst, all_batch)
    key = (cfg.N, cfg.E, _debug, tuple(shared["BA"].tolist()),
           tuple(shared["BB"].tolist()))
    if key not in _CACHE:
        _CACHE[key] = build(cfg, shared, debug=_debug)
    nc = _CACHE[key]
    wvals = _get_weights(kw, cfg)
    for m in in_maps:
        m.update(wvals)
    global _LAST_RESULT
    res = run_bass_kernel_spmd(nc, in_maps, core_ids=list(range(cfg.NCORES)),
                               trace=_trace)
    _LAST_RESULT = res
    return res.results[0]["out"].astype(np.float32)


def debug_run():
    import ml_dtypes as mld
    d = np.load('/tmp/inputs.npz')
    g = np.load('/tmp/gold_store.npz')
    cfg = Cfg()
    inputs = {k: d[k] for k in d.files}
    out = kernel(**inputs, _debug=True)
    res = _LAST_RESULT
    exp = np.load('/tmp/expected.npy')
    print("final rel:", np.abs(out - exp).max() / np.abs(exp).max())
    for ci in (0, 3):
        r = res.results[ci]
        h_hw = r['dbg_h'].view(mld.bfloat16)[:cfg.NLOC, 0:64].astype(np.float32)
        xe_hw = r['dbg_xe'].view(mld.bfloat16)[:, 0:64].astype(np.float32)
        zv_hw = r['dbg_zv'].view(mld.bfloat16).astype(np.float32)
        xT_hw = r['dbg_xT'].view(mld.bfloat16).astype(np.float32)
        # gold equivalents
        X = np.asarray(d['X'], np.float32)
        kwn = {k: np.asarray(d[k], np.float32) for k in
               "W_in b_in W1a b1a W1b b1b W2 b2 W3 b3 Wc1 bc1 Wc2 bc2".split()}
        x_in = np.maximum(X @ kwn['W_in'] + kwn['b_in'], 0)
        h_gold_full = np.maximum(x_in @ kwn['W1a'] + kwn['b1a'], 0) @ kwn['W1b'] + kwn['b1b']
        h_gold = h_gold_full[cfg.NLOC * ci:cfg.NLOC * (ci + 1)]
        xe_full = g['xe_0']
        L = xe_full[cfg.ESHL * ci:cfg.ESHL * (ci + 1)]
        H = xe_full[cfg.EWL * 128 + cfg.ESHH * ci:cfg.EWL * 128 + cfg.ESHH * (ci + 1)]
        xe_gold = np.concatenate([L, H], 0)
        zv_gold = g['zv_0'][cfg.NSH * ci:cfg.NSH * (ci + 1)]
        x_gold = g['x_0'][cfg.NLOC * ci:cfg.NLOC * (ci + 1)].T
        def cmp(name, a, b):
            s = max(np.abs(b).max(), 1e-9)
            print(f"  core{ci} {name}: rel={np.abs(a - b).max() / s:.3e} "
                  f"(scale {s:.2e})")
        cmp("h", h_hw, h_gold)
        cmp("xe", xe_hw, xe_gold)
        cmp("zv", zv_hw[:cfg.NLOC], zv_gold[:cfg.NLOC])
        cmp("xT", xT_hw, x_gold)


if __name__ == "__main__":
    debug_run()


# revision 32
# speedup vs baseline: 2.5078x; 1.0789x over previous
"""EquivSetGNN forward on 8 Trainium2 NeuronCores (Bass/Tile), v2.

Sharding/dataflow:
  V->E: entries partitioned by src (h table local per core), dst-sorted;
        per dst-window one-hot segment-sum matmuls into a partial Xe table
        (bf16, 128-wide padded rows); ReduceScatter gives each core the
        summed Xe rows for its 1/8 edge shard.
  E->V: entries partitioned by dst shard (Xe shard local), ordered by
        global node window; one-hot segment-sums into a partial Xv table
        over ALL nodes (core-aligned padded layout); ReduceScatter gives
        each core the summed Xv rows for its local nodes.
  Dense MLP phases run feature-transposed [64, n] with x/x0/tbbuf resident
  in SBUF (bf16); all dense matmuls bf16 (1 cycle/row).
  Readout: per-graph one-hot matmul sums + small AllReduce + 1/count.
"""
import sys

sys.path.insert(0, "/opt/trn_rl_repo")

import ml_dtypes
import numpy as np

import concourse.bass as bass
import concourse.bacc as bacc
import concourse.mybir as mybir
import concourse.tile as tile
from concourse.bass_utils import run_bass_kernel_spmd
from concourse.masks import make_identity
from contextlib import ExitStack

F32 = mybir.dt.float32
BF16 = mybir.dt.bfloat16
I16 = mybir.dt.int16
U32 = mybir.dt.uint32
AF = mybir.ActivationFunctionType
ALU = mybir.AluOpType
BF = ml_dtypes.bfloat16


class Cfg:
    def __init__(self, N=100000, E=50000, FT=128, HID=64, CLS_H=64, NCLS=32,
                 NGRAPH=256, NLAYER=2, NCORES=8):
        self.N, self.E, self.FT, self.HID = N, E, FT, HID
        self.CLS_H, self.NCLS, self.NGRAPH, self.NLAYER = CLS_H, NCLS, NGRAPH, NLAYER
        self.NCORES = NCORES
        assert N % NCORES == 0
        self.NLOC = N // NCORES              # 12500 local nodes
        self.NW = -(-self.NLOC // 128)       # 98 local node windows
        self.EW = -(-E // 128)               # 391 edge windows
        self.EPAD = self.EW * 128            # 50048
        assert self.EPAD % NCORES == 0
        self.ESH = self.EPAD // NCORES       # 6256 edge shard rows
        self.EWL = 256                       # V->E windows in first RS chunk
        self.ESHL = self.EWL * 128 // NCORES          # 3072
        self.ESHH = (self.EPAD - self.EWL * 128) // NCORES  # 3184
        self.NSH = 128 * (-(-self.NLOC // 128) * 128 // 128)  # padded local rows
        # pad local node count to a value s.t. 8*NSH is window-divisible
        self.NSH = -(-self.NLOC // 16) * 16  # 12512 (so NPAD=100096=782*128)
        self.NPAD = self.NSH * NCORES        # 100096
        self.NWG = self.NPAD // 128          # 782 global node windows
        assert self.NPAD % 128 == 0
        self.GW = -(-NGRAPH // 128)          # 2 graph windows


def _ceil(a, b):
    return -(-a // b)


def _wrap16(idx):
    """flat idx array -> [128, L/16] int16 wrapped layout (replicated x8)."""
    a = np.asarray(idx, np.int16).reshape(-1, 16).T
    return np.ascontiguousarray(np.tile(a, (8, 1)))


def _gath_layout(vals, dtype):
    """flat [L] -> [128, 1, L/128] layout (entry k at [k%128, 0, k//128])."""
    L = len(vals)
    assert L % 128 == 0
    return np.ascontiguousarray(
        np.asarray(vals, dtype).reshape(L // 128, 128).T.reshape(128, 1, L // 128))


def prep(cfg, X, v2e_src, v2e_dst, all_batch):
    """Host preprocessing -> (shared_structure, per_core_input_maps)."""
    c = cfg
    src = np.asarray(v2e_src, np.int64)
    dst = np.asarray(v2e_dst, np.int64)
    batch = np.asarray(all_batch, np.int64)

    d_deg = np.bincount(dst, minlength=c.E).astype(np.float32)
    c_deg = np.bincount(src, minlength=c.N).astype(np.float32)
    recip_d = (1.0 / np.maximum(d_deg, 1.0)).astype(np.float32)
    recip_c = (1.0 / np.maximum(c_deg, 1.0)).astype(np.float32)

    # ---- V->E stream: src-partitioned, dst-sorted ----
    cntA = np.zeros((c.NCORES, c.EW), np.int64)
    coreA = []
    for ci in range(c.NCORES):
        lo, hi = np.searchsorted(src, [c.NLOC * ci, c.NLOC * (ci + 1)])
        s = (src[lo:hi] - c.NLOC * ci).astype(np.int64)
        e = dst[lo:hi]
        order = np.argsort(e, kind="stable")
        sA, eA = s[order], e[order]
        win = eA >> 7
        cntA[ci] = np.bincount(win, minlength=c.EW)
        coreA.append((sA, eA, win))
    BA = -(-cntA.max(axis=0) // 128)
    capA = BA * 128
    offA = np.concatenate([[0], np.cumsum(capA)])
    LA = int(offA[-1])
    nblkA = LA // 128

    # ---- E->V stream: dst-shard partitioned, global-node-window ordered ----
    cntB = np.zeros((c.NCORES, c.NWG), np.int64)
    coreB = []
    owner = dst // c.ESH
    local_e = dst - c.ESH * owner
    for ci in range(c.NCORES):
        m = owner == ci
        sg = src[m]
        el = local_e[m]
        r = c.NSH * (sg // c.NLOC) + (sg % c.NLOC)
        win = r >> 7
        order = np.argsort(win, kind="stable")
        rB, eB, winB = r[order], el[order], win[order]
        cntB[ci] = np.bincount(winB, minlength=c.NWG)
        coreB.append((rB, eB, winB))
    BB = -(-cntB.max(axis=0) // 128)
    capB = BB * 128
    offB = np.concatenate([[0], np.cumsum(capB)])
    LB = int(offB[-1])
    nblkB = LB // 128

    shared = dict(BA=BA, BB=BB, LA=LA, LB=LB, nblkA=nblkA, nblkB=nblkB,
                  offA=offA, offB=offB)

    # graph readout constants
    gcnt = np.bincount(batch, minlength=c.NGRAPH).astype(np.float32)
    recip_g = (1.0 / np.maximum(gcnt, 1.0)).astype(np.float32)
    recip_g_win = np.zeros((128, c.GW), np.float32)
    for g in range(c.NGRAPH):
        recip_g_win[g % 128, g // 128] = recip_g[g]

    iota_e = np.broadcast_to(
        np.arange(128, dtype=BF)[None, :, None], (128, 128, 16))
    iota_e = np.ascontiguousarray(iota_e)

    in_maps = []
    for ci in range(c.NCORES):
        # V->E placement
        sA, eA, winA = coreA[ci]
        startsA = np.searchsorted(winA, np.arange(c.EW))
        place = offA[winA] + (np.arange(len(winA)) - startsA[winA])
        gidxA = np.zeros(LA, np.int64)
        idsA = np.full(LA, -1.0, np.float32)
        gidxA[place] = sA
        idsA[place] = (eA - (winA << 7)).astype(np.float32)

        # E->V placement
        rB, eB, winB = coreB[ci]
        startsB = np.searchsorted(winB, np.arange(c.NWG))
        place = offB[winB] + (np.arange(len(winB)) - startsB[winB])
        gidxB = np.zeros(LB, np.int64)
        idsB = np.full(LB, -1.0, np.float32)
        gidxB[place] = eB
        idsB[place] = (rB & 127).astype(np.float32)

        # per-local-node arrays in window layout [128, NW]
        npad = c.NW * 128
        cw = np.zeros(npad, np.float32)
        cw[:c.NLOC] = recip_c[c.NLOC * ci: c.NLOC * (ci + 1)]
        recip_c_win = np.ascontiguousarray(cw.reshape(c.NW, 128).T)
        mw = np.zeros(npad, np.float32)
        mw[:c.NLOC] = (c_deg[c.NLOC * ci: c.NLOC * (ci + 1)] > 0).astype(np.float32)
        mask_win = np.ascontiguousarray(mw.reshape(c.NW, 128).T)
        bw = np.full(npad, -1.0, np.float32)
        bw[:c.NLOC] = batch[c.NLOC * ci: c.NLOC * (ci + 1)].astype(np.float32)
        ids_g = np.ascontiguousarray(
            bw.reshape(c.NW, 128).T.astype(BF).reshape(128, 1, c.NW))
        bw1 = np.where(bw < 0, -1.0, bw - 128.0).astype(np.float32)
        ids_g1 = np.ascontiguousarray(
            bw1.reshape(c.NW, 128).T.astype(BF).reshape(128, 1, c.NW))

        dw = np.zeros(c.EW * 128, np.float32)
        dw[:c.E] = recip_d
        recip_d_win = np.ascontiguousarray(dw.reshape(c.EW, 128).T)

        m = {
            "Xs": np.ascontiguousarray(X[c.NLOC * ci: c.NLOC * (ci + 1)]).astype(np.float32),
            "gidxA": _wrap16(gidxA), "idsA": _gath_layout(idsA, BF),
            "gidxB": _wrap16(gidxB), "idsB": _gath_layout(idsB, BF),
            "recip_c_win": recip_c_win,
            "recip_d_win": recip_d_win, "ids_g": ids_g, "ids_g1": ids_g1,
            "recip_g_win": recip_g_win, "iota_e": iota_e,
        }
        in_maps.append(m)
    return shared, in_maps


def build(cfg, sh, debug=False):
    """Build the SPMD Bass program."""
    c = cfg
    nc = bacc.Bacc("TRN2", debug=False, num_swdge_queues=1)
    HID = c.HID
    nblkA, nblkB = sh["nblkA"], sh["nblkB"]

    # ---------- I/O ----------
    Xs = nc.declare_dram_parameter("Xs", [c.NLOC, c.FT], F32, isOutput=False)
    gidxA_d = nc.declare_dram_parameter("gidxA", [128, sh["LA"] // 16], I16, isOutput=False)
    idsA_d = nc.declare_dram_parameter("idsA", [128, 1, nblkA], BF16, isOutput=False)
    gidxB_d = nc.declare_dram_parameter("gidxB", [128, sh["LB"] // 16], I16, isOutput=False)
    idsB_d = nc.declare_dram_parameter("idsB", [128, 1, nblkB], BF16, isOutput=False)
    recip_c_d = nc.declare_dram_parameter("recip_c_win", [128, c.NW], F32, isOutput=False)
    recip_d_d = nc.declare_dram_parameter("recip_d_win", [128, c.EW], F32, isOutput=False)
    ids_g_d = nc.declare_dram_parameter("ids_g", [128, 1, c.NW], BF16, isOutput=False)
    ids_g1_d = nc.declare_dram_parameter("ids_g1", [128, 1, c.NW], BF16, isOutput=False)
    recip_g_d = nc.declare_dram_parameter("recip_g_win", [128, c.GW], F32, isOutput=False)
    iota_d = nc.declare_dram_parameter("iota_e", [128, 128, 16], BF16, isOutput=False)
    wdecl = {
        "W_in": ([c.FT, HID], BF16), "W1a": ([HID, HID], BF16),
        "W1b": ([HID, HID], BF16), "W2a": ([HID, HID], BF16),
        "W2b": ([HID, HID], BF16), "W3": ([HID, HID], BF16),
        "Wc1": ([HID, c.CLS_H], BF16), "Wc2": ([c.CLS_H, c.NCLS], BF16),
        "b_in": ([HID, 1], F32), "b1a": ([HID, 1], F32), "b1b": ([HID, 1], F32),
        "b3": ([HID, 1], F32), "bc1": ([c.CLS_H, 1], F32),
        "bc2_rep": ([128, c.NCLS], F32), "b2h": ([HID, 1], F32),
    }
    wparams = {k: nc.declare_dram_parameter(k, list(s), d, isOutput=False)
               for k, (s, d) in wdecl.items()}
    out_d = nc.declare_dram_parameter("out", [c.NGRAPH, c.NCLS], F32, isOutput=True)
    taps = {}
    if debug:
        taps["h"] = nc.declare_dram_parameter("dbg_h", [c.NW * 128, 128], BF16, isOutput=True)
        taps["xe"] = nc.declare_dram_parameter("dbg_xe", [c.ESH, 128], BF16, isOutput=True)
        taps["zv"] = nc.declare_dram_parameter("dbg_zv", [c.NSH, c.HID], BF16, isOutput=True)
        taps["xT"] = nc.declare_dram_parameter("dbg_xT", [c.HID, c.NLOC], BF16, isOutput=True)
        taps["tbbh"] = nc.declare_dram_parameter("dbg_tbbh", [c.HID, c.NLOC], BF16, isOutput=True)
        taps["xT1"] = nc.declare_dram_parameter("dbg_xT1", [c.HID, c.NLOC], BF16, isOutput=True)
        taps["zv1"] = nc.declare_dram_parameter("dbg_zv1", [c.NSH, c.HID], BF16, isOutput=True)
        taps["xe1"] = nc.declare_dram_parameter("dbg_xe1", [c.ESH, 128], BF16, isOutput=True)

    # ---------- internal DRAM ----------
    h_dram = nc.dram_tensor("h_tab", [c.NW * 128, 128], BF16)
    s_part = nc.dram_tensor("s_part", [c.EPAD, 128], BF16)
    xe_sh = nc.dram_tensor("xe_sh", [c.ESH, 128], BF16)
    xv_part = nc.dram_tensor("xv_part", [c.NPAD, HID], BF16)
    zv_sh = nc.dram_tensor("zv_sh", [c.NSH, HID], BF16)
    gsum_part = nc.dram_tensor("gsum_part", [c.GW * 128, c.NCLS], F32)
    gsum_full = nc.dram_tensor("gsum_full", [c.GW * 128, c.NCLS], F32,
                               addr_space="Shared")

    rg = [list(range(c.NCORES))]

    with tile.TileContext(nc) as tc:
        ctx = ExitStack()
        const = ctx.enter_context(tc.tile_pool(name="const", bufs=1))
        sb = ctx.enter_context(tc.tile_pool(name="sb", bufs=3))
        gp = ctx.enter_context(tc.tile_pool(name="gp", bufs=4))
        ohp = ctx.enter_context(tc.tile_pool(name="ohp", bufs=3))
        flp = ctx.enter_context(tc.tile_pool(name="flp", bufs=3))
        ps_big = ctx.enter_context(tc.tile_pool(name="ps_big", bufs=2, space="PSUM"))
        ps_dense = ctx.enter_context(tc.tile_pool(name="ps_dense", bufs=2, space="PSUM"))
        ps_g = ctx.enter_context(tc.tile_pool(name="ps_g", bufs=1, space="PSUM"))

        def load_const(dram, shape, dtype=F32):
            t = const.tile(shape, dtype, tag=f"c_{dram.name}")
            sl = tuple(slice(None) for _ in shape)
            nc.sync.dma_start(out=t[sl], in_=dram[sl])
            return t

        ident = const.tile([128, 128], F32)
        make_identity(nc, ident[:, :])
        ident_bf = const.tile([128, 128], BF16)
        nc.scalar.activation(out=ident_bf[:, :], in_=ident[:, :], func=AF.Copy)
        W = {k: load_const(v, wdecl[k][0], wdecl[k][1]) for k, v in wparams.items()}

        # resident activations
        xT = const.tile([HID, c.NLOC], BF16, tag="xT")
        x0h = const.tile([HID, c.NLOC], BF16, tag="x0h")
        tbbh = const.tile([HID, c.NLOC], BF16, tag="tbbh")

        CH = 512

        def chunks():
            o = 0
            while o < c.NLOC:
                yield o, min(CH, c.NLOC - o)
                o += CH

        def blkrows(m):
            return min(128, c.NLOC - 128 * m)

        # ---------- input layer: xT = relu(W_in^T @ X^T), x0h = 0.5 xT ------
        for o, n in chunks():
            xblk = sb.tile([128, 4, c.FT], F32, tag="xblk")
            if n % 128 == 0:
                nc.sync.dma_start(
                    out=xblk[:, :n // 128, :],
                    in_=Xs[o:o + n, :].rearrange("(j p) c -> p j c", p=128))
            else:
                for j in range(_ceil(n, 128)):
                    r = min(128, n - 128 * j)
                    nc.sync.dma_start(out=xblk[:r, j, :],
                                      in_=Xs[o + 128 * j:o + 128 * j + r, :])
            ptx = ps_big.tile([128, 512], F32, tag="pbig")
            for j in range(_ceil(n, 128)):
                r = min(128, n - 128 * j)
                nc.tensor.transpose(out=ptx[:c.FT, 128 * j:128 * j + r],
                                    in_=xblk[:r, j, :], identity=ident[:r, :r])
            xTb = sb.tile([128, 512], BF16, tag="xTb")
            nc.vector.tensor_scalar_mul(xTb[:, :n], ptx[:, :n], 1.0)
            pd = ps_dense.tile([HID, 512], F32, tag="pd")
            nc.tensor.matmul(out=pd[:, :n], lhsT=W["W_in"][:, :], rhs=xTb[:, :n],
                             start=True, stop=True)
            nc.scalar.activation(out=xT[:, o:o + n], in_=pd[:, :n], func=AF.Relu,
                                 bias=W["b_in"][:, 0:1])
            nc.vector.tensor_scalar_mul(x0h[:, o:o + n], xT[:, o:o + n], 0.5)

        iota_e = load_const(iota_d, [128, 128, 16], BF16)
        gidxA = load_const(gidxA_d, [128, sh["LA"] // 16], I16)
        idsA = load_const(idsA_d, [128, 1, nblkA], BF16)
        gidxB = load_const(gidxB_d, [128, sh["LB"] // 16], I16)
        idsB = load_const(idsB_d, [128, 1, nblkB], BF16)
        recip_c = load_const(recip_c_d, [128, c.NW])
        recip_dw = load_const(recip_d_d, [128, c.EW])
        ids_g = load_const(ids_g_d, [128, 1, c.NW], BF16)
        ids_g1 = load_const(ids_g1_d, [128, 1, c.NW], BF16)
        recip_gw = load_const(recip_g_d, [128, c.GW])

        BA, BB = sh["BA"], sh["BB"]
        offA, offB = sh["offA"], sh["offB"]

        def make_stream(idx_tile, ids_tile, src_ap, nblk_total, dtag):
            cache = {}

            def get(b):
                c0 = (b // 8) * 8
                if c0 not in cache:
                    nb = min(8, nblk_total - c0)
                    g = gp.tile([128, 8, 64], U32, tag=dtag)
                    nc.gpsimd.dma_gather(
                        out_ap=g[:, :nb, :], in_ap=src_ap.bitcast(U32),
                        idxs_ap=idx_tile[:, 8 * c0: 8 * c0 + 8 * nb],
                        num_idxs=128 * nb, num_idxs_reg=128 * nb, elem_size=64)
                    oh = ohp.tile([128, 128, 8], BF16, tag="oh" + dtag)
                    nc.vector.tensor_tensor(
                        out=oh[:, :, :nb],
                        in0=ids_tile[:, 0:1, c0:c0 + nb].to_broadcast([128, 128, nb]),
                        in1=iota_e[:, :, :nb], op=ALU.is_equal)
                    cache[c0] = (g, oh)
                g, oh = cache[c0]
                return g, oh, b - c0
            return get

        for layer in range(c.NLAYER):
            # ---------- h = relu(x@W1a+b1a)@W1b + b1b -> bf16 table ----------
            ps_h_cm = tc.tile_pool(name=f"ps_h{layer}", bufs=2, space="PSUM")
            ps_h = ps_h_cm.__enter__()
            for o, n in chunks():
                pd = ps_dense.tile([HID, 512], F32, tag="pd")
                nc.tensor.matmul(out=pd[:, :n], lhsT=W["W1a"][:, :], rhs=xT[:, o:o + n],
                                 start=True, stop=True)
                ut = sb.tile([HID, 512], BF16, tag="ut")
                nc.scalar.activation(out=ut[:, :n], in_=pd[:, :n], func=AF.Relu,
                                     bias=W["b1a"][:, 0:1])
                pd2 = ps_dense.tile([HID, 512], F32, tag="pd")
                nc.tensor.matmul(out=pd2[:, :n], lhsT=W["W1b"][:, :], rhs=ut[:, :n],
                                 start=True, stop=True)
                htb = sb.tile([HID, 512], BF16, tag="htb")
                nc.vector.tensor_scalar(htb[:, :n], pd2[:, :n], W["b1b"][:, 0:1],
                                        None, ALU.add)
                nb = _ceil(n, 128)
                ptb = ps_h.tile([128, 4, HID], BF16, tag="ptb")
                for j in range(nb):
                    nc.tensor.transpose(out=ptb[:, j, :],
                                        in_=htb[:, 128 * j:128 * (j + 1)],
                                        identity=ident_bf[:HID, :HID])
                hrm = flp.tile([128, 4, HID], BF16, tag="hrm")
                nc.scalar.activation(out=hrm[:, :nb, :],
                                     in_=ptb[:, :nb, :], func=AF.Copy)
                nc.sync.dma_start(
                    out=h_dram[o:o + 128 * nb, 0:HID].rearrange(
                        "(j p) c -> p j c", p=128),
                    in_=hrm[:, :nb, :])
            ps_h_cm.__exit__(None, None, None)

            if debug and layer == 0:
                nc.sync.dma_start(out=taps["h"][:, :], in_=h_dram[:, :])
            # ---------- V->E partials ----------
            ps_ve_cm = tc.tile_pool(name=f"ps_ve{layer}", bufs=2, space="PSUM")
            ps_ve = ps_ve_cm.__enter__()
            getA = make_stream(gidxA, idsA, h_dram[:, :], nblkA, "gA")
            cc1 = nc.alloc_semaphore(f"cc_xe{layer}")
            for w0 in range(0, c.EW, 8):
                wn = min(8, c.EW - w0)
                sfl = flp.tile([128, 8, HID], BF16, tag="sflA")
                for dw_ in range(wn):
                    w = w0 + dw_
                    nblk = int(BA[w])
                    if nblk == 0:
                        nc.vector.memset(sfl[:, dw_, :], 0.0)
                        continue
                    b0 = int(offA[w]) // 128
                    pw = ps_ve.tile([128, HID], F32, tag="pw")
                    for i in range(nblk):
                        g, oh, col = getA(b0 + i)
                        nc.tensor.matmul(out=pw[:, :], lhsT=oh[:, :, col],
                                         rhs=g[:, col, 0:HID // 2].bitcast(BF16),
                                         start=(i == 0), stop=(i == nblk - 1))
                    nc.scalar.activation(out=sfl[:, dw_, :], in_=pw[:, :],
                                         func=AF.Copy, scale=recip_dw[:, w:w + 1])
                nc.sync.dma_start(
                    out=s_part[128 * w0:128 * (w0 + wn), 0:HID].rearrange(
                        "(j p) c -> p j c", p=128),
                    in_=sfl[:, :wn, :])
            ps_ve_cm.__exit__(None, None, None)

            # ---------- ReduceScatter Xe ----------
            with tc.tile_critical():
                nc.gpsimd.collective_compute(
                    "ReduceScatter", ALU.add, replica_groups=rg,
                    ins=[s_part.ap().opt()], outs=[xe_sh.ap().opt()],
                ).then_inc(cc1, 1)

            # overlap: tbbh = 0.5*(x@W2a + b2) + x0h   (transposed space)
            for o, n in list(chunks())[:13]:
                pdp = ps_dense.tile([HID, 512], F32, tag="pd")
                nc.tensor.matmul(out=pdp[:, :n], lhsT=W["W2a"][:, :],
                                 rhs=xT[:, o:o + n], start=True, stop=True)
                nc.vector.scalar_tensor_tensor(
                    out=tbbh[:, o:o + n], in0=pdp[:, :n],
                    scalar=W["b2h"][:, 0:1], in1=x0h[:, o:o + n],
                    op0=ALU.add, op1=ALU.add)

            with tc.tile_critical():
                nc.gpsimd.wait_ge(cc1, 1)
            tc.strict_bb_all_engine_barrier()

            if debug:
                nc.sync.dma_start(out=taps["xe" if layer == 0 else "xe1"][:, :],
                                  in_=xe_sh[:, :])
            # ---------- E->V partials over global padded node windows -------
            getB = make_stream(gidxB, idsB, xe_sh[:, :], nblkB, "gB")
            for w0 in range(0, c.NWG, 8):
                wn = min(8, c.NWG - w0)
                pwf = ps_big.tile([128, 512], F32, tag="pbig")
                for dw_ in range(wn):
                    w = w0 + dw_
                    nblk = int(BB[w])
                    if nblk == 0:
                        nc.vector.memset(pwf[:, HID * dw_:HID * (dw_ + 1)], 0.0)
                        continue
                    b0 = int(offB[w]) // 128
                    for i in range(nblk):
                        g, oh, col = getB(b0 + i)
                        nc.tensor.matmul(out=pwf[:, HID * dw_:HID * (dw_ + 1)],
                                         lhsT=oh[:, :, col],
                                         rhs=g[:, col, 0:HID // 2].bitcast(BF16),
                                         start=(i == 0), stop=(i == nblk - 1))
                sfl2 = flp.tile([128, 8, HID], BF16, tag="sflB")
                nc.scalar.activation(
                    out=sfl2[:, :wn, :],
                    in_=pwf[:, :HID * wn].rearrange("p (j c) -> p j c", c=HID),
                    func=AF.Copy)
                nc.sync.dma_start(
                    out=xv_part[128 * w0:128 * (w0 + wn), :].rearrange(
                        "(j p) c -> p j c", p=128),
                    in_=sfl2[:, :wn, :])

            # ---------- ReduceScatter Xv ----------
            cc2 = nc.alloc_semaphore(f"cc_xv{layer}")
            with tc.tile_critical():
                nc.gpsimd.collective_compute(
                    "ReduceScatter", ALU.add, replica_groups=rg,
                    ins=[xv_part.ap().opt()], outs=[zv_sh.ap().opt()],
                ).then_inc(cc2, 1)
            for o, n in list(chunks())[13:]:
                pdp = ps_dense.tile([HID, 512], F32, tag="pd")
                nc.tensor.matmul(out=pdp[:, :n], lhsT=W["W2a"][:, :],
                                 rhs=xT[:, o:o + n], start=True, stop=True)
                nc.vector.scalar_tensor_tensor(
                    out=tbbh[:, o:o + n], in0=pdp[:, :n],
                    scalar=W["b2h"][:, 0:1], in1=x0h[:, o:o + n],
                    op0=ALU.add, op1=ALU.add)
            with tc.tile_critical():
                nc.gpsimd.wait_ge(cc2, 1)
            tc.strict_bb_all_engine_barrier()

            if debug:
                nc.sync.dma_start(out=taps["zv" if layer == 0 else "zv1"][:, :],
                                  in_=zv_sh[:, :])
                tbd = flp.tile([HID, 512], BF16, tag="tbd")
                for o, n in chunks():
                    nc.vector.tensor_scalar_mul(tbd[:, :n], tbbh[:, o:o + n], 1.0)
                    nc.sync.dma_start(out=taps["tbbh"][:, o:o + n], in_=tbd[:, :n])
            # ---------- dense update: x = relu((tbbh + (z*recip)@W2b_h) @ W3 + b3)
            ps_up_cm = tc.tile_pool(name=f"ps_up{layer}", bufs=2, space="PSUM")
            ps_up = ps_up_cm.__enter__()
            yt = None
            zv4 = None
            pzg = None
            for m in range(c.NW):
                rows = blkrows(m)
                if m % 4 == 0:
                    o4 = 128 * m
                    n4 = min(512, c.NLOC - o4)
                    yt = sb.tile([HID, 512], BF16, tag="yt")
                    pzg = ps_dense.tile([HID, 512], F32, tag="pd")
                    zv4 = sb.tile([128, 4, HID], BF16, tag="zv4")
                    if o4 + 512 <= c.NSH:
                        nc.sync.dma_start(
                            out=zv4[:, :, :],
                            in_=zv_sh[o4:o4 + 512, :].rearrange(
                                "(j p) c -> p j c", p=128))
                    else:
                        for j in range(_ceil(c.NSH - o4, 128)):
                            zr = min(128, c.NSH - o4 - 128 * j)
                            nc.sync.dma_start(
                                out=zv4[:zr, j, :],
                                in_=zv_sh[o4 + 128 * j:o4 + 128 * j + zr, :])
                co = 128 * m - o4
                if m % 4 == 0:
                    ptz4 = ps_up.tile([HID, 4, 128], BF16, tag="ptz")
                zs = flp.tile([128, HID], BF16, tag="zs")
                nc.vector.tensor_scalar(zs[:rows, :], zv4[:rows, m % 4, :],
                                        recip_c[:rows, m:m + 1], None, ALU.mult)
                nc.tensor.transpose(out=ptz4[:, m % 4, :rows], in_=zs[:rows, :],
                                    identity=ident_bf[:rows, :rows])
                if m % 4 == 3 or m == c.NW - 1:
                    nbw = m % 4 + 1
                    zts4 = sb.tile([HID, 4, 128], BF16, tag="zts")
                    nc.scalar.activation(out=zts4[:, :nbw, :], in_=ptz4[:, :nbw, :],
                                         func=AF.Copy)
                    for jj in range(nbw):
                        rw = min(128, c.NLOC - o4 - 128 * jj)
                        nc.tensor.matmul(out=pzg[:, 128 * jj:128 * jj + rw],
                                         lhsT=W["W2b"][:, :],
                                         rhs=zts4[:, jj, :rw], start=True, stop=True)
                    nc.vector.tensor_tensor(out=yt[:, :n4], in0=pzg[:, :n4],
                                            in1=tbbh[:, o4:o4 + n4], op=ALU.add)
                    pd3 = ps_dense.tile([HID, 512], F32, tag="pd")
                    nc.tensor.matmul(out=pd3[:, :n4], lhsT=W["W3"][:, :],
                                     rhs=yt[:, :n4], start=True, stop=True)
                    nc.scalar.activation(out=xT[:, o4:o4 + n4], in_=pd3[:, :n4],
                                         func=AF.Relu, bias=W["b3"][:, 0:1])
            ps_up_cm.__exit__(None, None, None)
            if debug and layer == 0:
                xtd0 = flp.tile([HID, 512], BF16, tag="xtd0")
                for o, n in chunks():
                    nc.vector.tensor_scalar_mul(xtd0[:, :n], xT[:, o:o + n], 1.0)
                    nc.sync.dma_start(out=taps["xT"][:, o:o + n], in_=xtd0[:, :n])

        if debug:
            xtd1 = flp.tile([HID, 512], BF16, tag="xtd1")
            for o, n in chunks():
                nc.vector.tensor_scalar_mul(xtd1[:, :n], xT[:, o:o + n], 1.0)
                nc.sync.dma_start(out=taps["xT1"][:, o:o + n], in_=xtd1[:, :n])
        # ---------- classifier + readout ----------
        ps_cl_cm = tc.tile_pool(name="ps_cl", bufs=2, space="PSUM")
        ps_cl = ps_cl_cm.__enter__()
        gps = []
        for g in range(c.GW):
            gtile = ps_g.tile([128, c.NCLS], F32, tag=f"gps{g}")
            gps.append(gtile)
        n_mm = [0, 0]
        for o, n in chunks():
            pd = ps_dense.tile([HID, 512], F32, tag="pd")
            nc.tensor.matmul(out=pd[:c.CLS_H, :n], lhsT=W["Wc1"][:, :],
                             rhs=xT[:, o:o + n], start=True, stop=True)
            ut = sb.tile([c.CLS_H, 512], BF16, tag="utc")
            nc.scalar.activation(out=ut[:, :n], in_=pd[:c.CLS_H, :n], func=AF.Relu,
                                 bias=W["bc1"][:, 0:1])
            nb = _ceil(n, 128)
            b0 = o // 128
            ohgs = []
            for gi, idst in ((0, ids_g), (1, ids_g1)):
                ohg = ohp.tile([128, 128, 4], BF16, tag=f"ohg{gi}")
                nc.vector.tensor_tensor(
                    out=ohg[:, :, :nb],
                    in0=idst[:, 0:1, b0:b0 + nb].to_broadcast([128, 128, nb]),
                    in1=iota_e[:, :, :nb], op=ALU.is_equal)
                ohgs.append(ohg)
            pcl4 = ps_cl.tile([128, 4, c.NCLS], F32, tag="pcls")
            for j in range(nb):
                r = min(128, n - 128 * j)
                nc.tensor.matmul(out=pcl4[:r, j, :], lhsT=ut[:, 128 * j:128 * j + r],
                                 rhs=W["Wc2"][:, :], start=True, stop=True)
            cls4 = flp.tile([128, 4, c.NCLS], BF16, tag="cls")
            nc.scalar.activation(out=cls4[:, :nb, :], in_=pcl4[:, :nb, :], func=AF.Copy)
            for j in range(nb):
                r = min(128, n - 128 * j)
                for gi in range(c.GW):
                    nc.tensor.matmul(out=gps[gi][:, :], lhsT=ohgs[gi][:r, :, j],
                                     rhs=cls4[:r, j, :],
                                     start=(n_mm[gi] == 0), stop=(n_mm[gi] == c.NW - 1))
                    n_mm[gi] += 1
        for g in range(c.GW):
            gfl = flp.tile([128, c.NCLS], F32, tag="gfl")
            nc.scalar.activation(out=gfl[:, :], in_=gps[g][:, :], func=AF.Copy)
            nc.sync.dma_start(out=gsum_part[128 * g:128 * (g + 1), :], in_=gfl[:, :])
        ps_cl_cm.__exit__(None, None, None)

        tc.strict_bb_all_engine_barrier()
        with tc.tile_critical():
            cc3 = nc.alloc_semaphore("cc_g")
            nc.gpsimd.collective_compute(
                "AllReduce", ALU.add, replica_groups=rg,
                ins=[gsum_part.ap().opt()], outs=[gsum_full.ap().opt()],
            ).then_inc(cc3, 1)
            nc.gpsimd.wait_ge(cc3, 1)
        tc.strict_bb_all_engine_barrier()

        for g in range(c.GW):
            gt = flp.tile([128, c.NCLS], F32, tag="gt")
            nc.sync.dma_start(out=gt[:, :], in_=gsum_full[128 * g:128 * (g + 1), :])
            go = flp.tile([128, c.NCLS], F32, tag="go")
            nc.vector.tensor_tensor(
                out=go[:, :], in0=gt[:, :],
                in1=recip_gw[:, g:g + 1].to_broadcast([128, c.NCLS]), op=ALU.mult)
            nc.vector.tensor_tensor(out=go[:, :], in0=go[:, :],
                                    in1=W["bc2_rep"][:, :], op=ALU.add)
            rows = min(128, c.NGRAPH - 128 * g)
            nc.sync.dma_start(out=out_d[128 * g:128 * g + rows, :], in_=go[:rows, :])
        ctx.close()

    nc.finalize()
    return nc


_CACHE = {}
_LAST_RESULT = None


def _get_weights(kw, cfg):
    """Host weight conversion; W2 split into 0.5*W2a/0.5*W2b; b2h = 0.5*b2."""
    W2 = np.asarray(kw["W2"], np.float32)
    vals = {
        "W_in": np.asarray(kw["W_in"], np.float32).astype(BF),
        "W1a": np.asarray(kw["W1a"], np.float32).astype(BF),
        "W1b": np.asarray(kw["W1b"], np.float32).astype(BF),
        "W2a": (0.5 * W2[:cfg.HID]).astype(BF), "W2b": (0.5 * W2[cfg.HID:]).astype(BF),
        "W3": np.asarray(kw["W3"], np.float32).astype(BF),
        "Wc1": np.asarray(kw["Wc1"], np.float32).astype(BF),
        "Wc2": np.asarray(kw["Wc2"], np.float32).astype(BF),
        "b_in": np.asarray(kw["b_in"], np.float32).reshape(-1, 1),
        "b1a": np.asarray(kw["b1a"], np.float32).reshape(-1, 1),
        "b1b": np.asarray(kw["b1b"], np.float32).reshape(-1, 1),
        "b3": np.asarray(kw["b3"], np.float32).reshape(-1, 1),
        "bc1": np.asarray(kw["bc1"], np.float32).reshape(-1, 1),
        "bc2_rep": np.tile(np.asarray(kw["bc2"], np.float32).reshape(1, -1), (128, 1)),
        "b2h": 0.5 * np.asarray(kw["b2"], np.float32).reshape(-1, 1),
    }
    return {k: np.ascontiguousarray(v) for k, v in vals.items()}


def kernel(X, v2e_src, v2e_dst, all_batch, W_in, b_in, W1a, b1a, W1b, b1b,
           W2, b2, W3, b3, Wc1, bc1, Wc2, bc2, _cfg=None, _trace=False,
           _debug=False):
    cfg = _cfg or Cfg()
    kw = dict(W_in=W_in, b_in=b_in, W1a=W1a, b1a=b1a, W1b=W1b, b1b=b1b, W2=W2,
              b2=b2, W3=W3, b3=b3, Wc1=Wc1, bc1=bc1, Wc2=Wc2, bc2=bc2)
    shared, in_maps = prep(cfg, np.asarray(X, np.float32), v2e_src, v2e_dst, all_batch)
    key = (cfg.N, cfg.E, _debug, tuple(shared["BA"].tolist()),
           tuple(shared["BB"].tolist()))
    if key not in _CACHE:
        _CACHE[key] = build(cfg, shared, debug=_debug)
    nc = _CACHE[key]
    wvals = _get_weights(kw, cfg)
    for m in in_maps:
        m.update(wvals)
    global _LAST_RESULT
    res = run_bass_kernel_spmd(nc, in_maps, core_ids=list(range(cfg.NCORES)),
                               trace=_trace)
    _LAST_RESULT = res
    return res.results[0]["out"].astype(np.float32)


def debug_run():
    import ml_dtypes as mld
    d = np.load('/tmp/inputs.npz')
    g = np.load('/tmp/gold_store.npz')
    cfg = Cfg()
    inputs = {k: d[k] for k in d.files}
    out = kernel(**inputs, _debug=True)
    res = _LAST_RESULT
    exp = np.load('/tmp/expected.npy')
    print("final rel:", np.abs(out - exp).max() / np.abs(exp).max())
    for ci in (0, 3):
        r = res.results[ci]
        h_hw = r['dbg_h'].view(mld.bfloat16)[:cfg.NLOC, 0:64].astype(np.float32)
        xe_hw = r['dbg_xe'].view(mld.bfloat16)[:, 0:64].astype(np.float32)
        zv_hw = r['dbg_zv'].view(mld.bfloat16).astype(np.float32)
        xT_hw = r['dbg_xT'].view(mld.bfloat16).astype(np.float32)
        # gold equivalents
        X = np.asarray(d['X'], np.float32)
        kwn = {k: np.asarray(d[k], np.float32) for k in
               "W_in b_in W1a b1a W1b b1b W2 b2 W3 b3 Wc1 bc1 Wc2 bc2".split()}
        x_in = np.maximum(X @ kwn['W_in'] + kwn['b_in'], 0)
        h_gold_full = np.maximum(x_in @ kwn['W1a'] + kwn['b1a'], 0) @ kwn['W1b'] + kwn['b1b']
        h_gold = h_gold_full[cfg.NLOC * ci:cfg.NLOC * (ci + 1)]
        xe_full = g['xe_0']
        L = xe_full[cfg.ESHL * ci:cfg.ESHL * (ci + 1)]
        H = xe_full[cfg.EWL * 128 + cfg.ESHH * ci:cfg.EWL * 128 + cfg.ESHH * (ci + 1)]
        xe_gold = np.concatenate([L, H], 0)
        zv_gold = g['zv_0'][cfg.NSH * ci:cfg.NSH * (ci + 1)]
        x_gold = g['x_0'][cfg.NLOC * ci:cfg.NLOC * (ci + 1)].T
        def cmp(name, a, b):
            s = max(np.abs(b).max(), 1e-9)
            print(f"  core{ci} {name}: rel={np.abs(a - b).max() / s:.3e} "
                  f"(scale {s:.2e})")
        cmp("h", h_hw, h_gold)
        cmp("xe", xe_hw, xe_gold)
        cmp("zv", zv_hw[:cfg.NLOC], zv_gold[:cfg.NLOC])
        cmp("xT", xT_hw, x_gold)


if __name__ == "__main__":
    debug_run()
